# revision 12
# baseline (speedup 1.0000x reference)
"""Single-head attention (B=4, S=4096, E=1024, H=64) on 8 TRN2 NeuronCores.

Sharding: core c -> (batch b = c//2, sequence half h = c%2). Each core receives
only its own 2048-row x half, computes Q/K/V for it, and the core pair
(2b, 2b+1) exchanges K/V halves with a 2-rank AllGather (two chunked AGs,
overlapped with the projection and the first attention tiles). Every core then
holds K/V for the full 4096-row sequence in global order and computes
attention for its 2048 queries.

Matmuls run in bf16 (fp32 lowers to two LOW_HIGH PE passes on TRN2 — half
throughput); accumulation is fp32 in PSUM, the softmax denominator and the
normalization stay fp32. All matmuls are zero-padded to full 128x128
stationary tiles: masked sub-tile matmuls (K=64 / M=65) leave the PE
clock-gated at 1.2 GHz (HAM does not see them as activity), while full tiles
keep it at 2.4 GHz; the padding costs no extra stream cycles.

Output projection: W_out is padded with b_out as row 64 and the bf16 context
carries the softmax denominator in row 64, so (ctx_aug.T @ W_out_aug) *
recip(denom) applies scale and bias in one pass (denom * recip == 1).

Changes over the original two-phase version, from trace analysis:
- the phase-B constants (kt zero rows, v_aug ones, W_out staging) are emitted
  BEFORE the AllGather readouts on the gpsimd queue: they used to sit behind
  the readout that waits for the last AG (~80us), gating the first scores
  matmul at ~83us; now attention starts as soon as AG0's readout lands.
- the second query chunk runs attention+output-projection in two 512-wide
  subchunks: the exposed serial tail after the last context matmul (which ran
  at 1.2 GHz because the HAM clock gate re-throttles during the ~4us scalar
  chain) shrinks by half.
- the scalar (ACT) engine does only exp in the attention region; casts,
  PSUM copies, and output scaling run on DVE so exp is never queued behind
  them (exp is the attention-phase rate limiter at ~1.15us per [128,1024]).
"""

import sys

import numpy as np

for _p in ("/opt/trn_rl_repo",):
    if _p not in sys.path:
        sys.path.insert(0, _p)

from contextlib import ExitStack

import concourse.bass as bass  # noqa: F401  (import keeps bass registered)
import concourse.mybir as mybir
import concourse.tile as tile
from concourse import bacc, masks
from concourse.bass_utils import run_bass_kernel_spmd

F32 = mybir.dt.float32
BF16 = mybir.dt.bfloat16
AF = mybir.ActivationFunctionType
ALU = mybir.AluOpType

B, S, E, H = 4, 4096, 1024, 64
D3 = 3 * H            # 192
SH = S // 2           # queries per core
N_CORES = 8
QC = 1024             # first query chunk (PSUM-sized)
ST = S // 128         # 32 kj tiles over the full sequence
ETILES = E // 128     # 8 embedding tiles
WSTR = 256            # w_sb per-e-tile stride: [K|Q|V|0] columns
CW = 512              # phase-A chunk width (columns of the own half)
NCH = SH // CW        # 4 chunks, one AllGather each
SCALE = 0.125         # 1/sqrt(H)
WKV = 64 * CW * 2     # AG payload elems per chunk: kT[64,CW] + vT[64,CW]
REPLICA_GROUPS = [[0, 1], [2, 3], [4, 5], [6, 7]]


def _emit(nc, tc, x_ext, wq_ext, bq_ext, wo_ext, bo_ext, out_ext):
    with ExitStack() as top:
        const = top.enter_context(tc.tile_pool(name="const", bufs=1))

        # Critical path first: identity (needed by the first transposes) and
        # the QKV weight staging.
        ident = const.tile([128, 128], BF16)
        masks.make_identity(nc, ident[:])

        # Weights: DMA fp32 staging -> cast to bf16.
        wstage_ctx = ExitStack()
        wstage = wstage_ctx.enter_context(tc.tile_pool(name="wstage", bufs=1))
        w32 = wstage.tile([128, ETILES * D3], F32)
        nc.gpsimd.dma_start(
            w32[:].rearrange("p (e d) -> p e d", d=D3),
            wq_ext.rearrange("(e p) d -> p e d", p=128),
        )
        w_sb = const.tile([128, ETILES * WSTR], BF16)
        w_sb_v = w_sb[:].rearrange("p (e c) -> p e c", c=WSTR)
        w32_v = w32[:].rearrange("p (e c) -> p e c", c=D3)
        nc.vector.tensor_copy(w_sb_v[:, :, 0:64], w32_v[:, :, 64:128])     # K
        nc.vector.tensor_copy(w_sb_v[:, :, 64:128], w32_v[:, :, 0:64])     # Q
        nc.vector.tensor_copy(w_sb_v[:, :, 128:192], w32_v[:, :, 128:192])  # V

        bkq = const.tile([128, 1], F32)  # [b_k ; b_q]
        nc.gpsimd.dma_start(bkq[0:64, :], bq_ext[64:128].unsqueeze(1))
        nc.gpsimd.dma_start(bkq[64:128, :], bq_ext[0:64].unsqueeze(1))
        bv = const.tile([64, 1], F32)
        nc.gpsimd.dma_start(bv[:], bq_ext[128:192].unsqueeze(1))

        # Persistent bf16 matmul operands (global kv order on the free axis)
        kt_sb = const.tile([128, S], BF16)   # kT on 0:64, zeros on 64:128
        vT_sb = const.tile([64, S], BF16)
        q2_sb = const.tile([128, SH], BF16)  # qT on 0:64, zeros on 64:128
        nc.gpsimd.memset(q2_sb[:], 0.0)
        v_aug = const.tile([128, ST * 128], BF16)
        ones11 = const.tile([1, 1], BF16)
        wo_sb = const.tile([128, E], BF16)

        def emit_consts():
            # phase-B constants: emitted on gpsimd between AG issues — late
            # enough not to delay AG0's trigger, early enough to be ordered
            # before the AG readouts (which gate the first scores matmul)
            nc.gpsimd.memset(kt_sb[64:128, :], 0.0)
            nc.gpsimd.memset(ones11[:], 1.0)
            nc.gpsimd.memset(
                v_aug[:].rearrange("p (t c) -> p t c", c=128)[:, :, 64:65], 1.0
            )
            wo32 = wstage.tile([H, E], F32)
            nc.gpsimd.dma_start(wo32[:], wo_ext[:, :])
            bo32 = wstage.tile([1, E], F32)
            nc.gpsimd.dma_start(bo32[:], bo_ext.unsqueeze(0))
            bo16 = wstage.tile([1, E], BF16)
            nc.vector.tensor_copy(bo16[:], bo32[:])
            nc.gpsimd.memset(wo_sb[:], 0.0)
            nc.vector.tensor_copy(wo_sb[0:64, :], wo32[:])
            nc.gpsimd.dma_start(wo_sb[64:65, :], bo16[:])

        # Small PSUM pool shared by phase-A v-transposes and phase-C tiles
        mps = top.enter_context(tc.tile_pool(name="mps", bufs=2, space="PSUM"))

        # Collective bounce buffers (per AG chunk)
        dram = top.enter_context(tc.tile_pool(name="ccdram", bufs=1, space="DRAM"))
        cc_in = [dram.tile([1, WKV], BF16, name=f"cc_in{c}") for c in range(NCH)]
        cc_out = [dram.tile([2, WKV], BF16, name=f"cc_out{c}") for c in range(NCH)]

        # ---- Phase A: per own-half s-chunk: cast, PE-transpose, project,
        # stage K/V into the pair AllGather --------------------------------
        with ExitStack() as pa:
            xsb = pa.enter_context(tc.tile_pool(name="xsb", bufs=12))
            xbp = pa.enter_context(tc.tile_pool(name="xbp", bufs=8))
            xTp = pa.enter_context(tc.tile_pool(name="xTp", bufs=3))
            stg = pa.enter_context(tc.tile_pool(name="stg", bufs=5))
            kqs = []
            xtp = pa.enter_context(tc.tile_pool(name="xtp", bufs=2, space="PSUM"))
            m1p = pa.enter_context(tc.tile_pool(name="m1p", bufs=2, space="PSUM"))
            m2p = pa.enter_context(tc.tile_pool(name="m2p", bufs=2, space="PSUM"))

            for sc in range(NCH):              # own-half s chunks of CW cols
                xbs = []
                for si in range(CW // 128):
                    st = sc * (CW // 128) + si
                    t32 = xsb.tile([128, E], F32)
                    # split each tile across both HWDGE queues and cast each
                    # half as soon as it lands
                    nc.sync.dma_start(
                        t32[0:64, :], x_ext[st * 128 : st * 128 + 64, :]
                    )
                    nc.scalar.dma_start(
                        t32[64:128, :], x_ext[st * 128 + 64 : (st + 1) * 128, :]
                    )
                    tb = xbp.tile([128, E], BF16)
                    nc.vector.tensor_copy(tb[:], t32[:])
                    xbs.append(tb)
                xT_sc = xTp.tile([128, ETILES * CW], BF16)
                for e in range(ETILES):
                    p = xtp.tile([128, CW], F32)
                    for si in range(CW // 128):
                        nc.tensor.matmul(
                            p[:, si * 128 : (si + 1) * 128],
                            xbs[si][:, e * 128 : (e + 1) * 128],
                            ident[:],
                        )
                    nc.vector.tensor_copy(xT_sc[:, e * CW : (e + 1) * CW], p[:])

                m1 = m1p.tile([128, CW], F32)
                m2 = m2p.tile([128, CW], F32)
                for e in range(ETILES):
                    lhs1 = w_sb[:, e * WSTR : e * WSTR + 128]
                    lhs2 = w_sb[:, e * WSTR + 128 : e * WSTR + 256]
                    rhs = xT_sc[:, e * CW : (e + 1) * CW]
                    nc.tensor.matmul(
                        m1[:], lhs1, rhs,
                        start=(e == 0), stop=(e == ETILES - 1),
                    )
                    nc.tensor.matmul(
                        m2[:], lhs2, rhs,
                        start=(e == 0), stop=(e == ETILES - 1),
                    )
                kq = stg.tile([128, CW], BF16, tag="kq")
                kqs.append(kq)
                nc.vector.tensor_scalar_add(kq[:], m1[:], bkq[:])
                vst = stg.tile([64, CW], BF16, tag="vst")
                nc.vector.tensor_scalar_add(vst[:], m2[0:64, :], bv[:])

                # stage into the AG (gpsimd stream only)
                nc.gpsimd.dma_start(cc_in[sc][0, 0 : 64 * CW], kq[0:64, :])
                nc.gpsimd.dma_start(cc_in[sc][0, 64 * CW : WKV], vst[:])
                nc.gpsimd.collective_compute(
                    "AllGather",
                    ALU.bypass,
                    replica_groups=REPLICA_GROUPS,
                    ins=[cc_in[sc].opt()],
                    outs=[cc_out[sc].opt()],
                )
                if sc == 1:
                    emit_consts()

            # q2 moves and AG readouts AFTER every CC issue, all on the
            # gpsimd stream
            for sc in range(NCH):
                nc.gpsimd.dma_start(
                    q2_sb[0:64, sc * CW : (sc + 1) * CW], kqs[sc][64:128, :]
                )
            for sc in range(NCH):
                for r in range(2):
                    cols = slice(r * SH + sc * CW, r * SH + (sc + 1) * CW)
                    nc.gpsimd.dma_start(
                        kt_sb[0:64, cols],
                        cc_out[sc][r, 0 : 64 * CW].rearrange("(p f) -> p f", p=64),
                    )
                    nc.gpsimd.dma_start(
                        vT_sb[:, cols],
                        cc_out[sc][r, 64 * CW : WKV].rearrange("(p f) -> p f", p=64),
                    )
        wstage_ctx.close()

        # kj visit order: tiles in AG-chunk completion order
        kpc = CW // 128  # kj tiles per AG chunk per half
        kj_order = []
        for c in range(NCH):
            kj_order += list(range(c * kpc, (c + 1) * kpc))
            kj_order += list(range(16 + c * kpc, 16 + (c + 1) * kpc))

        # ---- Phase B/C: attention + output projection -------------------
        # first chunk at QC=1024 (best exp amortization), second chunk as
        # two 512-wide subchunks so the exposed end-of-kernel tail is short
        with ExitStack() as pb:
            sps = pb.enter_context(tc.tile_pool(name="sps", bufs=2, space="PSUM"))
            cps = pb.enter_context(tc.tile_pool(name="cps", bufs=1, space="PSUM"))
            expp = pb.enter_context(tc.tile_pool(name="expp", bufs=8))
            ctxp = pb.enter_context(tc.tile_pool(name="ctxp", bufs=2))
            rsp = pb.enter_context(tc.tile_pool(name="rsp", bufs=2))
            outp = pb.enter_context(tc.tile_pool(name="outp", bufs=4))

            first_visit = [True]

            def emit_attn(q0, w):
                """attention for queries [q0, q0+w); returns ctx PSUM tile"""
                ctx = cps.tile([128, QC], F32, name="ctx", tag="ctx")
                for i, kj in enumerate(kj_order):
                    sc_ps = sps.tile([128, QC], F32, name="sc_ps", tag="sc")
                    lhs_k = kt_sb[:, kj * 128 : (kj + 1) * 128]
                    for n0 in range(0, w, 512):
                        nw = min(512, w - n0)
                        nc.tensor.matmul(
                            sc_ps[:, n0 : n0 + nw],
                            lhs_k,
                            q2_sb[:, q0 + n0 : q0 + n0 + nw],
                        )
                    ex = expp.tile([128, QC], BF16, name="ex", tag="ex")
                    nc.scalar.activation(
                        ex[:, 0:w], sc_ps[:, 0:w], AF.Exp, scale=SCALE
                    )
                    if first_visit[0]:  # v natural tile, first use
                        p = mps.tile([128, 64], F32, tag="mp", name="vtp")
                        nc.tensor.matmul(
                            p[:],
                            vT_sb[:, kj * 128 : (kj + 1) * 128],
                            ident[0:64, 0:64],
                        )
                        nc.vector.tensor_copy(
                            v_aug[:, kj * 128 : kj * 128 + 64], p[:]
                        )
                    lhs_v = v_aug[:, kj * 128 : (kj + 1) * 128]
                    for n0 in range(0, w, 512):
                        nw = min(512, w - n0)
                        nc.tensor.matmul(
                            ctx[:, n0 : n0 + nw],
                            lhs_v,
                            ex[:, n0 : n0 + nw],
                            start=(i == 0), stop=(i == ST - 1),
                            skip_group_check=True,
                        )
                first_visit[0] = False
                return ctx

            def emit_phc(ctx, q0, w):
                """output projection for queries [q0, q0+w)"""
                ctx_sb = ctxp.tile([65, QC], F32, tag="ctx32", name="ctx_sb")
                nc.vector.tensor_copy(ctx_sb[:, 0:w], ctx[0:65, 0:w])
                ctx_b16 = ctxp.tile([128, QC], BF16, tag="ctx16", name="ctx_b16")
                nc.gpsimd.memset(ctx_b16[64:128, 0:w], 0.0)
                nc.vector.tensor_copy(ctx_b16[0:65, 0:w], ctx_sb[:, 0:w])
                rs_row = rsp.tile([1, QC], BF16, tag="rsrow", name="rs_row")
                nc.vector.tensor_copy(rs_row[:, 0:w], ctx_b16[64:65, 0:w])

                rs_ps = mps.tile([128, QC // 128], F32, tag="mp", name="rsps")
                for c in range(w // 128):
                    nc.tensor.matmul(
                        rs_ps[:, c : c + 1],
                        rs_row[0:1, c * 128 : (c + 1) * 128],
                        ones11[:],
                    )
                recip = rsp.tile([128, QC // 128], F32, tag="recip", name="recip")
                nc.vector.reciprocal(recip[:, 0 : w // 128], rs_ps[:, 0 : w // 128])

                for c in range(w // 128):
                    out_sb = outp.tile([128, E], F32, name="out_sb")
                    for n in range(2):
                        op = mps.tile([128, 512], F32, tag="mp", name="opps")
                        nc.tensor.matmul(
                            op[:],
                            ctx_b16[:, c * 128 : (c + 1) * 128],
                            wo_sb[:, n * 512 : (n + 1) * 512],
                        )
                        nc.vector.tensor_scalar_mul(
                            out_sb[:, n * 512 : (n + 1) * 512],
                            op[:],
                            recip[:, c : c + 1],
                        )
                    (nc.sync if c % 2 == 0 else nc.scalar).dma_start(
                        out_ext[q0 + c * 128 : q0 + (c + 1) * 128, :], out_sb[:]
                    )

            ctx0 = emit_attn(0, 1024)
            emit_phc(ctx0, 0, 1024)
            ctx1 = emit_attn(1024, 512)
            emit_phc(ctx1, 1024, 512)
            ctx2 = emit_attn(1536, 256)
            emit_phc(ctx2, 1536, 256)
            ctx3 = emit_attn(1792, 256)
            emit_phc(ctx3, 1792, 256)


_NC = None


def _get_nc():
    global _NC
    if _NC is None:
        nc = bacc.Bacc("TRN2", target_bir_lowering=False, debug=False,
                       num_devices=N_CORES)
        x_ext = nc.dram_tensor("x", [SH, E], F32, kind="ExternalInput").ap()
        wq_ext = nc.dram_tensor("w_qkv", [E, D3], F32, kind="ExternalInput").ap()
        bq_ext = nc.dram_tensor("b_qkv", [D3], F32, kind="ExternalInput").ap()
        wo_ext = nc.dram_tensor("w_out", [H, E], F32, kind="ExternalInput").ap()
        bo_ext = nc.dram_tensor("b_out", [E], F32, kind="ExternalInput").ap()
        out_ext = nc.dram_tensor("out", [SH, E], F32, kind="ExternalOutput").ap()
        with tile.TileContext(nc) as tc:
            _emit(nc, tc, x_ext, wq_ext, bq_ext, wo_ext, bo_ext, out_ext)
        nc.compile()
        _NC = nc
    return _NC


last_results = None
last_tmpdir = None


def kernel(x, W_qkv, b_qkv, W_out, b_out):
    nc = _get_nc()
    x = np.ascontiguousarray(x, dtype=np.float32)
    shared = {
        "w_qkv": np.ascontiguousarray(W_qkv, dtype=np.float32),
        "b_qkv": np.ascontiguousarray(b_qkv, dtype=np.float32),
        "w_out": np.ascontiguousarray(W_out, dtype=np.float32),
        "b_out": np.ascontiguousarray(b_out, dtype=np.float32),
    }
    in_maps = []
    for c in range(N_CORES):
        b, h = divmod(c, 2)
        xp = np.ascontiguousarray(x[b, h * SH : (h + 1) * SH])
        in_maps.append({"x": xp, **shared})

    import os
    import tempfile
    import time

    tmpdir = os.environ.get("ATTN_TRACE_DIR") or tempfile.mkdtemp(prefix="attn_trace_")
    res = None
    for attempt in range(3):
        try:
            res = run_bass_kernel_spmd(
                nc, in_maps, core_ids=list(range(N_CORES)), tmpdir=tmpdir
            )
            break
        except Exception:
            # transient NRT_EXEC_UNIT_UNRECOVERABLE has been observed on a
            # first attempt; a clean retry recovers
            if attempt == 2:
                raise
            time.sleep(2.0)
    global last_results, last_tmpdir
    last_results = res
    last_tmpdir = tmpdir

    out = np.empty((B, S, E), dtype=np.float32)
    for c in range(N_CORES):
        b, h = divmod(c, 2)
        out[b, h * SH : (h + 1) * SH] = res.results[c]["out"]
    return out


# revision 14
# speedup vs baseline: 1.0031x; 1.0031x over previous
"""Single-head attention (B=4, S=4096, E=1024, H=64) on 8 TRN2 NeuronCores.

Sharding: core c -> (batch b = c//2, sequence half h = c%2). Each core receives
only its own 2048-row x half, computes Q/K/V for it, and the core pair
(2b, 2b+1) exchanges K/V halves with a 2-rank AllGather (two chunked AGs,
overlapped with the projection and the first attention tiles). Every core then
holds K/V for the full 4096-row sequence in global order and computes
attention for its 2048 queries.

Matmuls run in bf16 (fp32 lowers to two LOW_HIGH PE passes on TRN2 — half
throughput); accumulation is fp32 in PSUM, the softmax denominator and the
normalization stay fp32. All matmuls are zero-padded to full 128x128
stationary tiles: masked sub-tile matmuls (K=64 / M=65) leave the PE
clock-gated at 1.2 GHz (HAM does not see them as activity), while full tiles
keep it at 2.4 GHz; the padding costs no extra stream cycles.

Output projection: W_out is padded with b_out as row 64 and the bf16 context
carries the softmax denominator in row 64, so (ctx_aug.T @ W_out_aug) *
recip(denom) applies scale and bias in one pass (denom * recip == 1).

Changes over the original two-phase version, from trace analysis:
- the phase-B constants (kt zero rows, v_aug ones, W_out staging) are emitted
  BEFORE the AllGather readouts on the gpsimd queue: they used to sit behind
  the readout that waits for the last AG (~80us), gating the first scores
  matmul at ~83us; now attention starts as soon as AG0's readout lands.
- the second query chunk runs attention+output-projection in two 512-wide
  subchunks: the exposed serial tail after the last context matmul (which ran
  at 1.2 GHz because the HAM clock gate re-throttles during the ~4us scalar
  chain) shrinks by half.
- the scalar (ACT) engine does only exp in the attention region; casts,
  PSUM copies, and output scaling run on DVE so exp is never queued behind
  them (exp is the attention-phase rate limiter at ~1.15us per [128,1024]).
"""

import sys

import numpy as np

for _p in ("/opt/trn_rl_repo",):
    if _p not in sys.path:
        sys.path.insert(0, _p)

from contextlib import ExitStack

import concourse.bass as bass  # noqa: F401  (import keeps bass registered)
import concourse.mybir as mybir
import concourse.tile as tile
from concourse import bacc, masks
from concourse.bass_utils import run_bass_kernel_spmd

F32 = mybir.dt.float32
BF16 = mybir.dt.bfloat16
AF = mybir.ActivationFunctionType
ALU = mybir.AluOpType

B, S, E, H = 4, 4096, 1024, 64
D3 = 3 * H            # 192
SH = S // 2           # queries per core
N_CORES = 8
QC = 1024             # first query chunk (PSUM-sized)
ST = S // 128         # 32 kj tiles over the full sequence
ETILES = E // 128     # 8 embedding tiles
WSTR = 256            # w_sb per-e-tile stride: [K|Q|V|0] columns
CW = 512              # phase-A chunk width (columns of the own half)
NCH = SH // CW        # 4 chunks, one AllGather each
SCALE = 0.125         # 1/sqrt(H)
WKV = 64 * CW * 2     # AG payload elems per chunk: kT[64,CW] + vT[64,CW]
REPLICA_GROUPS = [[0, 1], [2, 3], [4, 5], [6, 7]]


def _emit(nc, tc, x_ext, wq_ext, bq_ext, wo_ext, bo_ext, out_ext):
    with ExitStack() as top:
        const = top.enter_context(tc.tile_pool(name="const", bufs=1))

        # Critical path first: identity (needed by the first transposes) and
        # the QKV weight staging.
        ident = const.tile([128, 128], BF16)
        masks.make_identity(nc, ident[:])

        # Weights: DMA fp32 staging -> cast to bf16.
        wstage_ctx = ExitStack()
        wstage = wstage_ctx.enter_context(tc.tile_pool(name="wstage", bufs=1))
        w32 = wstage.tile([128, ETILES * D3], F32)
        nc.gpsimd.dma_start(
            w32[:].rearrange("p (e d) -> p e d", d=D3),
            wq_ext.rearrange("(e p) d -> p e d", p=128),
        )
        w_sb = const.tile([128, ETILES * WSTR], BF16)
        w_sb_v = w_sb[:].rearrange("p (e c) -> p e c", c=WSTR)
        w32_v = w32[:].rearrange("p (e c) -> p e c", c=D3)
        nc.vector.tensor_copy(w_sb_v[:, :, 0:64], w32_v[:, :, 64:128])     # K
        nc.vector.tensor_copy(w_sb_v[:, :, 64:128], w32_v[:, :, 0:64])     # Q
        nc.vector.tensor_copy(w_sb_v[:, :, 128:192], w32_v[:, :, 128:192])  # V

        bkq = const.tile([128, 1], F32)  # [b_k ; b_q]
        nc.gpsimd.dma_start(bkq[0:64, :], bq_ext[64:128].unsqueeze(1))
        nc.gpsimd.dma_start(bkq[64:128, :], bq_ext[0:64].unsqueeze(1))
        bv = const.tile([64, 1], F32)
        nc.gpsimd.dma_start(bv[:], bq_ext[128:192].unsqueeze(1))

        # Persistent bf16 matmul operands (global kv order on the free axis)
        kt_sb = const.tile([128, S], BF16)   # kT on 0:64, zeros on 64:128
        vT_sb = const.tile([64, S], BF16)
        q2_sb = const.tile([128, SH], BF16)  # qT on 0:64, zeros on 64:128
        nc.gpsimd.memset(q2_sb[:], 0.0)
        v_aug = const.tile([128, ST * 128], BF16)
        ones11 = const.tile([1, 1], BF16)
        wo_sb = const.tile([128, E], BF16)

        # Phase-B constants EARLY (they used to sit behind the AG readouts on
        # the gpsimd queue and gated the first scores matmul by ~12us)
        nc.gpsimd.memset(kt_sb[64:128, :], 0.0)
        nc.gpsimd.memset(ones11[:], 1.0)
        nc.gpsimd.memset(
            v_aug[:].rearrange("p (t c) -> p t c", c=128)[:, :, 64:65], 1.0
        )
        wo32 = wstage.tile([H, E], F32)
        nc.gpsimd.dma_start(wo32[:], wo_ext[:, :])
        bo32 = wstage.tile([1, E], F32)
        nc.gpsimd.dma_start(bo32[:], bo_ext.unsqueeze(0))
        bo16 = wstage.tile([1, E], BF16)
        nc.vector.tensor_copy(bo16[:], bo32[:])
        nc.gpsimd.memset(wo_sb[:], 0.0)
        nc.vector.tensor_copy(wo_sb[0:64, :], wo32[:])
        nc.gpsimd.dma_start(wo_sb[64:65, :], bo16[:])

        # Small PSUM pool shared by phase-A v-transposes and phase-C tiles
        mps = top.enter_context(tc.tile_pool(name="mps", bufs=2, space="PSUM"))

        # Collective bounce buffers (per AG chunk)
        dram = top.enter_context(tc.tile_pool(name="ccdram", bufs=1, space="DRAM"))
        cc_in = [dram.tile([1, WKV], BF16, name=f"cc_in{c}") for c in range(NCH)]
        cc_out = [dram.tile([2, WKV], BF16, name=f"cc_out{c}") for c in range(NCH)]

        # ---- Phase A: per own-half s-chunk: cast, PE-transpose, project,
        # stage K/V into the pair AllGather --------------------------------
        with ExitStack() as pa:
            xsb = pa.enter_context(tc.tile_pool(name="xsb", bufs=12))
            xbp = pa.enter_context(tc.tile_pool(name="xbp", bufs=8))
            xTp = pa.enter_context(tc.tile_pool(name="xTp", bufs=3))
            stg = pa.enter_context(tc.tile_pool(name="stg", bufs=5))
            kqs = []
            xtp = pa.enter_context(tc.tile_pool(name="xtp", bufs=2, space="PSUM"))
            m1p = pa.enter_context(tc.tile_pool(name="m1p", bufs=2, space="PSUM"))
            m2p = pa.enter_context(tc.tile_pool(name="m2p", bufs=2, space="PSUM"))

            for sc in range(NCH):              # own-half s chunks of CW cols
                xbs = []
                for si in range(CW // 128):
                    st = sc * (CW // 128) + si
                    t32 = xsb.tile([128, E], F32)
                    # split each tile across both HWDGE queues and cast each
                    # half as soon as it lands
                    nc.sync.dma_start(
                        t32[0:64, :], x_ext[st * 128 : st * 128 + 64, :]
                    )
                    nc.scalar.dma_start(
                        t32[64:128, :], x_ext[st * 128 + 64 : (st + 1) * 128, :]
                    )
                    tb = xbp.tile([128, E], BF16)
                    nc.vector.tensor_copy(tb[:], t32[:])
                    xbs.append(tb)
                xT_sc = xTp.tile([128, ETILES * CW], BF16)
                for e in range(ETILES):
                    p = xtp.tile([128, CW], F32)
                    for si in range(CW // 128):
                        nc.tensor.matmul(
                            p[:, si * 128 : (si + 1) * 128],
                            xbs[si][:, e * 128 : (e + 1) * 128],
                            ident[:],
                        )
                    nc.vector.tensor_copy(xT_sc[:, e * CW : (e + 1) * CW], p[:])

                m1 = m1p.tile([128, CW], F32)
                m2 = m2p.tile([128, CW], F32)
                for e in range(ETILES):
                    lhs1 = w_sb[:, e * WSTR : e * WSTR + 128]
                    lhs2 = w_sb[:, e * WSTR + 128 : e * WSTR + 256]
                    rhs = xT_sc[:, e * CW : (e + 1) * CW]
                    nc.tensor.matmul(
                        m1[:], lhs1, rhs,
                        start=(e == 0), stop=(e == ETILES - 1),
                    )
                    nc.tensor.matmul(
                        m2[:], lhs2, rhs,
                        start=(e == 0), stop=(e == ETILES - 1),
                    )
                kq = stg.tile([128, CW], BF16, tag="kq")
                kqs.append(kq)
                nc.vector.tensor_scalar_add(kq[:], m1[:], bkq[:])
                vst = stg.tile([64, CW], BF16, tag="vst")
                nc.vector.tensor_scalar_add(vst[:], m2[0:64, :], bv[:])

                # stage into the AG (gpsimd stream only)
                nc.gpsimd.dma_start(cc_in[sc][0, 0 : 64 * CW], kq[0:64, :])
                nc.gpsimd.dma_start(cc_in[sc][0, 64 * CW : WKV], vst[:])
                nc.gpsimd.collective_compute(
                    "AllGather",
                    ALU.bypass,
                    replica_groups=REPLICA_GROUPS,
                    ins=[cc_in[sc].opt()],
                    outs=[cc_out[sc].opt()],
                )

            # q2 moves and AG readouts AFTER every CC issue, all on the
            # gpsimd stream
            for sc in range(NCH):
                nc.gpsimd.dma_start(
                    q2_sb[0:64, sc * CW : (sc + 1) * CW], kqs[sc][64:128, :]
                )
            for sc in range(NCH):
                for r in range(2):
                    cols = slice(r * SH + sc * CW, r * SH + (sc + 1) * CW)
                    nc.gpsimd.dma_start(
                        kt_sb[0:64, cols],
                        cc_out[sc][r, 0 : 64 * CW].rearrange("(p f) -> p f", p=64),
                    )
                    nc.gpsimd.dma_start(
                        vT_sb[:, cols],
                        cc_out[sc][r, 64 * CW : WKV].rearrange("(p f) -> p f", p=64),
                    )
        wstage_ctx.close()

        # kj visit order: tiles in AG-chunk completion order
        kpc = CW // 128  # kj tiles per AG chunk per half
        kj_order = []
        for c in range(NCH):
            kj_order += list(range(c * kpc, (c + 1) * kpc))
            kj_order += list(range(16 + c * kpc, 16 + (c + 1) * kpc))

        # ---- Phase B/C: attention + output projection -------------------
        # first chunk at QC=1024 (best exp amortization), second chunk as
        # two 512-wide subchunks so the exposed end-of-kernel tail is short
        with ExitStack() as pb:
            sps = pb.enter_context(tc.tile_pool(name="sps", bufs=2, space="PSUM"))
            cps = pb.enter_context(tc.tile_pool(name="cps", bufs=1, space="PSUM"))
            expp = pb.enter_context(tc.tile_pool(name="expp", bufs=8))
            ctxp = pb.enter_context(tc.tile_pool(name="ctxp", bufs=2))
            rsp = pb.enter_context(tc.tile_pool(name="rsp", bufs=2))
            outp = pb.enter_context(tc.tile_pool(name="outp", bufs=4))

            first_visit = [True]

            def emit_attn(q0, w):
                """attention for queries [q0, q0+w); returns ctx PSUM tile"""
                ctx = cps.tile([128, QC], F32, name="ctx", tag="ctx")
                for i, kj in enumerate(kj_order):
                    sc_ps = sps.tile([128, QC], F32, name="sc_ps", tag="sc")
                    lhs_k = kt_sb[:, kj * 128 : (kj + 1) * 128]
                    for n in range(w // 512):
                        nc.tensor.matmul(
                            sc_ps[:, n * 512 : (n + 1) * 512],
                            lhs_k,
                            q2_sb[:, q0 + n * 512 : q0 + (n + 1) * 512],
                        )
                    ex = expp.tile([128, QC], BF16, name="ex", tag="ex")
                    nc.scalar.activation(
                        ex[:, 0:w], sc_ps[:, 0:w], AF.Exp, scale=SCALE
                    )
                    if first_visit[0]:  # v natural tile, first use
                        p = mps.tile([128, 64], F32, tag="mp", name="vtp")
                        nc.tensor.matmul(
                            p[:],
                            vT_sb[:, kj * 128 : (kj + 1) * 128],
                            ident[0:64, 0:64],
                        )
                        nc.vector.tensor_copy(
                            v_aug[:, kj * 128 : kj * 128 + 64], p[:]
                        )
                    lhs_v = v_aug[:, kj * 128 : (kj + 1) * 128]
                    for n in range(w // 512):
                        nc.tensor.matmul(
                            ctx[:, n * 512 : (n + 1) * 512],
                            lhs_v,
                            ex[:, n * 512 : (n + 1) * 512],
                            start=(i == 0), stop=(i == ST - 1),
                            skip_group_check=True,
                        )
                first_visit[0] = False
                return ctx

            def emit_phc(ctx, q0, w):
                """output projection for queries [q0, q0+w)"""
                ctx_sb = ctxp.tile([65, QC], F32, tag="ctx32", name="ctx_sb")
                nc.vector.tensor_copy(ctx_sb[:, 0:w], ctx[0:65, 0:w])
                ctx_b16 = ctxp.tile([128, QC], BF16, tag="ctx16", name="ctx_b16")
                nc.gpsimd.memset(ctx_b16[64:128, 0:w], 0.0)
                nc.vector.tensor_copy(ctx_b16[0:65, 0:w], ctx_sb[:, 0:w])
                rs_row = rsp.tile([1, QC], BF16, tag="rsrow", name="rs_row")
                nc.sync.dma_start(rs_row[:, 0:w], ctx_b16[64:65, 0:w])

                rs_ps = mps.tile([128, QC // 128], F32, tag="mp", name="rsps")
                for c in range(w // 128):
                    nc.tensor.matmul(
                        rs_ps[:, c : c + 1],
                        rs_row[0:1, c * 128 : (c + 1) * 128],
                        ones11[:],
                    )
                recip = rsp.tile([128, QC // 128], F32, tag="recip", name="recip")
                nc.vector.reciprocal(recip[:, 0 : w // 128], rs_ps[:, 0 : w // 128])

                for c in range(w // 128):
                    out_sb = outp.tile([128, E], F32, name="out_sb")
                    for n in range(2):
                        op = mps.tile([128, 512], F32, tag="mp", name="opps")
                        nc.tensor.matmul(
                            op[:],
                            ctx_b16[:, c * 128 : (c + 1) * 128],
                            wo_sb[:, n * 512 : (n + 1) * 512],
                        )
                        nc.vector.tensor_scalar_mul(
                            out_sb[:, n * 512 : (n + 1) * 512],
                            op[:],
                            recip[:, c : c + 1],
                        )
                    (nc.sync if c % 2 == 0 else nc.scalar).dma_start(
                        out_ext[q0 + c * 128 : q0 + (c + 1) * 128, :], out_sb[:]
                    )

            ctx0 = emit_attn(0, 1024)
            emit_phc(ctx0, 0, 1024)
            ctx1 = emit_attn(1024, 512)
            emit_phc(ctx1, 1024, 512)
            ctx2 = emit_attn(1536, 512)
            emit_phc(ctx2, 1536, 512)


_NC = None


def _get_nc():
    global _NC
    if _NC is None:
        nc = bacc.Bacc("TRN2", target_bir_lowering=False, debug=False,
                       num_devices=N_CORES)
        x_ext = nc.dram_tensor("x", [SH, E], F32, kind="ExternalInput").ap()
        wq_ext = nc.dram_tensor("w_qkv", [E, D3], F32, kind="ExternalInput").ap()
        bq_ext = nc.dram_tensor("b_qkv", [D3], F32, kind="ExternalInput").ap()
        wo_ext = nc.dram_tensor("w_out", [H, E], F32, kind="ExternalInput").ap()
        bo_ext = nc.dram_tensor("b_out", [E], F32, kind="ExternalInput").ap()
        out_ext = nc.dram_tensor("out", [SH, E], F32, kind="ExternalOutput").ap()
        with tile.TileContext(nc) as tc:
            _emit(nc, tc, x_ext, wq_ext, bq_ext, wo_ext, bo_ext, out_ext)
        nc.compile()
        _NC = nc
    return _NC


last_results = None
last_tmpdir = None


def kernel(x, W_qkv, b_qkv, W_out, b_out):
    nc = _get_nc()
    x = np.ascontiguousarray(x, dtype=np.float32)
    shared = {
        "w_qkv": np.ascontiguousarray(W_qkv, dtype=np.float32),
        "b_qkv": np.ascontiguousarray(b_qkv, dtype=np.float32),
        "w_out": np.ascontiguousarray(W_out, dtype=np.float32),
        "b_out": np.ascontiguousarray(b_out, dtype=np.float32),
    }
    in_maps = []
    for c in range(N_CORES):
        b, h = divmod(c, 2)
        xp = np.ascontiguousarray(x[b, h * SH : (h + 1) * SH])
        in_maps.append({"x": xp, **shared})

    import os
    import tempfile
    import time

    tmpdir = os.environ.get("ATTN_TRACE_DIR") or tempfile.mkdtemp(prefix="attn_trace_")
    res = None
    for attempt in range(3):
        try:
            res = run_bass_kernel_spmd(
                nc, in_maps, core_ids=list(range(N_CORES)), tmpdir=tmpdir
            )
            break
        except Exception:
            # transient NRT_EXEC_UNIT_UNRECOVERABLE has been observed on a
            # first attempt; a clean retry recovers
            if attempt == 2:
                raise
            time.sleep(2.0)
    global last_results, last_tmpdir
    last_results = res
    last_tmpdir = tmpdir

    out = np.empty((B, S, E), dtype=np.float32)
    for c in range(N_CORES):
        b, h = divmod(c, 2)
        out[b, h * SH : (h + 1) * SH] = res.results[c]["out"]
    return out


# revision 15
# speedup vs baseline: 1.0459x; 1.0427x over previous
"""Single-head attention (B=4, S=4096, E=1024, H=64) on 8 TRN2 NeuronCores.

Sharding: core c -> (batch b = c//2, sequence half h = c%2). Each core receives
only its own 2048-row x half, computes Q/K/V for it, and the core pair
(2b, 2b+1) exchanges K/V halves with a 2-rank AllGather (two chunked AGs,
overlapped with the projection and the first attention tiles). Every core then
holds K/V for the full 4096-row sequence in global order and computes
attention for its 2048 queries.

Matmuls run in bf16 (fp32 lowers to two LOW_HIGH PE passes on TRN2 — half
throughput); accumulation is fp32 in PSUM, the softmax denominator and the
normalization stay fp32. All matmuls are zero-padded to full 128x128
stationary tiles: masked sub-tile matmuls (K=64 / M=65) leave the PE
clock-gated at 1.2 GHz (HAM does not see them as activity), while full tiles
keep it at 2.4 GHz; the padding costs no extra stream cycles.

Output projection: W_out is padded with b_out as row 64 and the bf16 context
carries the softmax denominator in row 64, so (ctx_aug.T @ W_out_aug) *
recip(denom) applies scale and bias in one pass (denom * recip == 1).

Changes over the original two-phase version, from trace analysis:
- the phase-B constants (kt zero rows, v_aug ones, W_out staging) are emitted
  BEFORE the AllGather readouts on the gpsimd queue: they used to sit behind
  the readout that waits for the last AG (~80us), gating the first scores
  matmul at ~83us; now attention starts as soon as AG0's readout lands.
- the second query chunk runs attention+output-projection in two 512-wide
  subchunks: the exposed serial tail after the last context matmul (which ran
  at 1.2 GHz because the HAM clock gate re-throttles during the ~4us scalar
  chain) shrinks by half.
- the scalar (ACT) engine does only exp in the attention region; casts,
  PSUM copies, and output scaling run on DVE so exp is never queued behind
  them (exp is the attention-phase rate limiter at ~1.15us per [128,1024]).
"""

import sys

import numpy as np

for _p in ("/opt/trn_rl_repo",):
    if _p not in sys.path:
        sys.path.insert(0, _p)

from contextlib import ExitStack

import concourse.bass as bass  # noqa: F401  (import keeps bass registered)
import concourse.mybir as mybir
import concourse.tile as tile
from concourse import bacc, masks
from concourse.bass_utils import run_bass_kernel_spmd

F32 = mybir.dt.float32
BF16 = mybir.dt.bfloat16
AF = mybir.ActivationFunctionType
ALU = mybir.AluOpType

B, S, E, H = 4, 4096, 1024, 64
D3 = 3 * H            # 192
SH = S // 2           # queries per core
N_CORES = 8
QC = 1024             # first query chunk (PSUM-sized)
ST = S // 128         # 32 kj tiles over the full sequence
ETILES = E // 128     # 8 embedding tiles
WSTR = 256            # w_sb per-e-tile stride: [K|Q|V|0] columns
CW = 512              # phase-A chunk width (columns of the own half)
NCH = SH // CW        # 4 chunks, one AllGather each
SCALE = 0.125         # 1/sqrt(H)
WKV = 64 * CW * 2     # AG payload elems per chunk: kT[64,CW] + vT[64,CW]
REPLICA_GROUPS = [[0, 1], [2, 3], [4, 5], [6, 7]]


def _emit(nc, tc, x_ext, wq_ext, bq_ext, wo_ext, bo_ext, out_ext):
    with ExitStack() as top:
        const = top.enter_context(tc.tile_pool(name="const", bufs=1))

        # Critical path first: identity (needed by the first transposes) and
        # the QKV weight staging.
        ident = const.tile([128, 128], BF16)
        masks.make_identity(nc, ident[:])

        # Weights: DMA fp32 staging -> cast to bf16.
        wstage_ctx = ExitStack()
        wstage = wstage_ctx.enter_context(tc.tile_pool(name="wstage", bufs=1))
        w32 = wstage.tile([128, ETILES * D3], F32)
        nc.gpsimd.dma_start(
            w32[:].rearrange("p (e d) -> p e d", d=D3),
            wq_ext.rearrange("(e p) d -> p e d", p=128),
        )
        w_sb = const.tile([128, ETILES * WSTR], BF16)
        w_sb_v = w_sb[:].rearrange("p (e c) -> p e c", c=WSTR)
        w32_v = w32[:].rearrange("p (e c) -> p e c", c=D3)
        nc.vector.tensor_copy(w_sb_v[:, :, 0:64], w32_v[:, :, 64:128])     # K
        nc.vector.tensor_copy(w_sb_v[:, :, 64:128], w32_v[:, :, 0:64])     # Q
        nc.vector.tensor_copy(w_sb_v[:, :, 128:192], w32_v[:, :, 128:192])  # V

        # v_aug fully zeroed on DVE so ctx PSUM rows 65:128 are exact zeros
        # (enables a single full-width bf16 context copy in phase C)
        v_aug = const.tile([128, ST * 128], BF16)
        nc.vector.memset(v_aug[:], 0.0)
        nc.vector.memset(
            v_aug[:].rearrange("p (t c) -> p t c", c=128)[:, :, 64:65], 1.0
        )

        bkq = const.tile([128, 1], F32)  # [b_k ; b_q]
        nc.gpsimd.dma_start(bkq[0:64, :], bq_ext[64:128].unsqueeze(1))
        nc.gpsimd.dma_start(bkq[64:128, :], bq_ext[0:64].unsqueeze(1))
        bv = const.tile([64, 1], F32)
        nc.gpsimd.dma_start(bv[:], bq_ext[128:192].unsqueeze(1))

        # Persistent bf16 matmul operands (global kv order on the free axis)
        kt_sb = const.tile([128, S], BF16)   # kT on 0:64, zeros on 64:128
        vT_sb = const.tile([64, S], BF16)
        q2_sb = const.tile([128, SH], BF16)  # qT on 0:64, zeros on 64:128
        nc.gpsimd.memset(q2_sb[:], 0.0)
        ones11 = const.tile([1, 1], BF16)
        wo_sb = const.tile([128, E], BF16)

        # Phase-B constants EARLY (they used to sit behind the AG readouts on
        # the gpsimd queue and gated the first scores matmul by ~12us)
        nc.gpsimd.memset(kt_sb[64:128, :], 0.0)
        nc.gpsimd.memset(ones11[:], 1.0)
        wo32 = wstage.tile([H, E], F32)
        nc.gpsimd.dma_start(wo32[:], wo_ext[:, :])
        bo32 = wstage.tile([1, E], F32)
        nc.gpsimd.dma_start(bo32[:], bo_ext.unsqueeze(0))
        bo16 = wstage.tile([1, E], BF16)
        nc.vector.tensor_copy(bo16[:], bo32[:])
        nc.gpsimd.memset(wo_sb[:], 0.0)
        nc.vector.tensor_copy(wo_sb[0:64, :], wo32[:])
        nc.gpsimd.dma_start(wo_sb[64:65, :], bo16[:])

        # Small PSUM pool shared by phase-A v-transposes and phase-C tiles
        mps = top.enter_context(tc.tile_pool(name="mps", bufs=2, space="PSUM"))

        # Collective bounce buffers (per AG chunk)
        dram = top.enter_context(tc.tile_pool(name="ccdram", bufs=1, space="DRAM"))
        cc_in = [dram.tile([1, WKV], BF16, name=f"cc_in{c}") for c in range(NCH)]
        cc_out = [dram.tile([2, WKV], BF16, name=f"cc_out{c}") for c in range(NCH)]

        # ---- Phase A: per own-half s-chunk: cast, PE-transpose, project,
        # stage K/V into the pair AllGather --------------------------------
        with ExitStack() as pa:
            xsb = pa.enter_context(tc.tile_pool(name="xsb", bufs=12))
            xbp = pa.enter_context(tc.tile_pool(name="xbp", bufs=8))
            xTp = pa.enter_context(tc.tile_pool(name="xTp", bufs=3))
            stg = pa.enter_context(tc.tile_pool(name="stg", bufs=5))
            kqs = []
            xtp = pa.enter_context(tc.tile_pool(name="xtp", bufs=2, space="PSUM"))
            m1p = pa.enter_context(tc.tile_pool(name="m1p", bufs=2, space="PSUM"))
            m2p = pa.enter_context(tc.tile_pool(name="m2p", bufs=2, space="PSUM"))

            for sc in range(NCH):              # own-half s chunks of CW cols
                xbs = []
                for si in range(CW // 128):
                    st = sc * (CW // 128) + si
                    t32 = xsb.tile([128, E], F32)
                    # split each tile across both HWDGE queues and cast each
                    # half as soon as it lands
                    nc.sync.dma_start(
                        t32[0:64, :], x_ext[st * 128 : st * 128 + 64, :]
                    )
                    nc.scalar.dma_start(
                        t32[64:128, :], x_ext[st * 128 + 64 : (st + 1) * 128, :]
                    )
                    tb = xbp.tile([128, E], BF16)
                    nc.vector.tensor_copy(tb[:], t32[:])
                    xbs.append(tb)
                xT_sc = xTp.tile([128, ETILES * CW], BF16)
                for e in range(ETILES):
                    p = xtp.tile([128, CW], F32)
                    for si in range(CW // 128):
                        nc.tensor.matmul(
                            p[:, si * 128 : (si + 1) * 128],
                            xbs[si][:, e * 128 : (e + 1) * 128],
                            ident[:],
                        )
                    nc.vector.tensor_copy(xT_sc[:, e * CW : (e + 1) * CW], p[:])

                m1 = m1p.tile([128, CW], F32)
                m2 = m2p.tile([128, CW], F32)
                for e in range(ETILES):
                    lhs1 = w_sb[:, e * WSTR : e * WSTR + 128]
                    lhs2 = w_sb[:, e * WSTR + 128 : e * WSTR + 256]
                    rhs = xT_sc[:, e * CW : (e + 1) * CW]
                    nc.tensor.matmul(
                        m1[:], lhs1, rhs,
                        start=(e == 0), stop=(e == ETILES - 1),
                    )
                    nc.tensor.matmul(
                        m2[:], lhs2, rhs,
                        start=(e == 0), stop=(e == ETILES - 1),
                    )
                kq = stg.tile([128, CW], BF16, tag="kq")
                kqs.append(kq)
                nc.vector.tensor_scalar_add(kq[:], m1[:], bkq[:])
                vst = stg.tile([64, CW], BF16, tag="vst")
                nc.vector.tensor_scalar_add(vst[:], m2[0:64, :], bv[:])

                # stage into the AG (gpsimd stream only)
                nc.gpsimd.dma_start(cc_in[sc][0, 0 : 64 * CW], kq[0:64, :])
                nc.gpsimd.dma_start(cc_in[sc][0, 64 * CW : WKV], vst[:])
                nc.gpsimd.collective_compute(
                    "AllGather",
                    ALU.bypass,
                    replica_groups=REPLICA_GROUPS,
                    ins=[cc_in[sc].opt()],
                    outs=[cc_out[sc].opt()],
                )

            # q2 moves and AG readouts AFTER every CC issue, all on the
            # gpsimd stream
            for sc in range(NCH):
                nc.gpsimd.dma_start(
                    q2_sb[0:64, sc * CW : (sc + 1) * CW], kqs[sc][64:128, :]
                )
            for sc in range(NCH):
                for r in range(2):
                    cols = slice(r * SH + sc * CW, r * SH + (sc + 1) * CW)
                    nc.gpsimd.dma_start(
                        kt_sb[0:64, cols],
                        cc_out[sc][r, 0 : 64 * CW].rearrange("(p f) -> p f", p=64),
                    )
                    nc.gpsimd.dma_start(
                        vT_sb[:, cols],
                        cc_out[sc][r, 64 * CW : WKV].rearrange("(p f) -> p f", p=64),
                    )
        wstage_ctx.close()

        # kj visit order: tiles in AG-chunk completion order
        kpc = CW // 128  # kj tiles per AG chunk per half
        kj_order = []
        for c in range(NCH):
            kj_order += list(range(c * kpc, (c + 1) * kpc))
            kj_order += list(range(16 + c * kpc, 16 + (c + 1) * kpc))

        # ---- Phase B/C: attention + output projection -------------------
        # first chunk at QC=1024 (best exp amortization), second chunk as
        # two 512-wide subchunks so the exposed end-of-kernel tail is short
        with ExitStack() as pb:
            sps = pb.enter_context(tc.tile_pool(name="sps", bufs=2, space="PSUM"))
            cps = pb.enter_context(tc.tile_pool(name="cps", bufs=1, space="PSUM"))
            expp = pb.enter_context(tc.tile_pool(name="expp", bufs=8))
            ctxp = pb.enter_context(tc.tile_pool(name="ctxp", bufs=2))
            rsp = pb.enter_context(tc.tile_pool(name="rsp", bufs=2))
            outp = pb.enter_context(tc.tile_pool(name="outp", bufs=4))

            first_visit = [True]

            def emit_attn(q0, w):
                """attention for queries [q0, q0+w); returns ctx PSUM tile"""
                ctx = cps.tile([128, QC], F32, name="ctx", tag="ctx")
                for i, kj in enumerate(kj_order):
                    sc_ps = sps.tile([128, QC], F32, name="sc_ps", tag="sc")
                    lhs_k = kt_sb[:, kj * 128 : (kj + 1) * 128]
                    for n in range(w // 512):
                        nc.tensor.matmul(
                            sc_ps[:, n * 512 : (n + 1) * 512],
                            lhs_k,
                            q2_sb[:, q0 + n * 512 : q0 + (n + 1) * 512],
                        )
                    ex = expp.tile([128, QC], BF16, name="ex", tag="ex")
                    nc.scalar.activation(
                        ex[:, 0:w], sc_ps[:, 0:w], AF.Exp, scale=SCALE
                    )
                    if first_visit[0]:  # v natural tile, first use
                        p = mps.tile([128, 64], F32, tag="mp", name="vtp")
                        nc.tensor.matmul(
                            p[:],
                            vT_sb[:, kj * 128 : (kj + 1) * 128],
                            ident[0:64, 0:64],
                        )
                        nc.vector.tensor_copy(
                            v_aug[:, kj * 128 : kj * 128 + 64], p[:]
                        )
                    lhs_v = v_aug[:, kj * 128 : (kj + 1) * 128]
                    for n in range(w // 512):
                        nc.tensor.matmul(
                            ctx[:, n * 512 : (n + 1) * 512],
                            lhs_v,
                            ex[:, n * 512 : (n + 1) * 512],
                            start=(i == 0), stop=(i == ST - 1),
                            skip_group_check=True,
                        )
                first_visit[0] = False
                return ctx

            def emit_phc(ctx, q0, w):
                """output projection for queries [q0, q0+w)"""
                # rows 65:128 of ctx are exact zeros (v_aug zero padding), so
                # one full-width bf16 copy suffices: the serial chain stays
                # under the ~3.4us HAM window and the out matmuls run warm
                ctx_b16 = ctxp.tile([128, QC], BF16, tag="ctx16", name="ctx_b16")
                nc.vector.tensor_copy(ctx_b16[:, 0:w], ctx[:, 0:w])
                rs_row = rsp.tile([1, QC], BF16, tag="rsrow", name="rs_row")
                nc.vector.tensor_copy(rs_row[:, 0:w], ctx_b16[64:65, 0:w])

                rs_ps = mps.tile([128, QC // 128], F32, tag="mp", name="rsps")
                for c in range(w // 128):
                    nc.tensor.matmul(
                        rs_ps[:, c : c + 1],
                        rs_row[0:1, c * 128 : (c + 1) * 128],
                        ones11[:],
                    )
                recip = rsp.tile([128, QC // 128], F32, tag="recip", name="recip")
                nc.vector.reciprocal(recip[:, 0 : w // 128], rs_ps[:, 0 : w // 128])

                for c in range(w // 128):
                    out_sb = outp.tile([128, E], F32, name="out_sb")
                    for n in range(2):
                        op = mps.tile([128, 512], F32, tag="mp", name="opps")
                        nc.tensor.matmul(
                            op[:],
                            ctx_b16[:, c * 128 : (c + 1) * 128],
                            wo_sb[:, n * 512 : (n + 1) * 512],
                        )
                        nc.vector.tensor_scalar_mul(
                            out_sb[:, n * 512 : (n + 1) * 512],
                            op[:],
                            recip[:, c : c + 1],
                        )
                    # sync only: a scalar-queue DMA issue costs ~0.6us of
                    # ACT time that would delay the exp stream
                    nc.sync.dma_start(
                        out_ext[q0 + c * 128 : q0 + (c + 1) * 128, :], out_sb[:]
                    )

            ctx0 = emit_attn(0, 1024)
            emit_phc(ctx0, 0, 1024)
            ctx1 = emit_attn(1024, 1024)
            emit_phc(ctx1, 1024, 1024)


_NC = None


def _get_nc():
    global _NC
    if _NC is None:
        nc = bacc.Bacc("TRN2", target_bir_lowering=False, debug=False,
                       num_devices=N_CORES)
        x_ext = nc.dram_tensor("x", [SH, E], F32, kind="ExternalInput").ap()
        wq_ext = nc.dram_tensor("w_qkv", [E, D3], F32, kind="ExternalInput").ap()
        bq_ext = nc.dram_tensor("b_qkv", [D3], F32, kind="ExternalInput").ap()
        wo_ext = nc.dram_tensor("w_out", [H, E], F32, kind="ExternalInput").ap()
        bo_ext = nc.dram_tensor("b_out", [E], F32, kind="ExternalInput").ap()
        out_ext = nc.dram_tensor("out", [SH, E], F32, kind="ExternalOutput").ap()
        with tile.TileContext(nc) as tc:
            _emit(nc, tc, x_ext, wq_ext, bq_ext, wo_ext, bo_ext, out_ext)
        nc.compile()
        _NC = nc
    return _NC


last_results = None
last_tmpdir = None


def kernel(x, W_qkv, b_qkv, W_out, b_out):
    nc = _get_nc()
    x = np.ascontiguousarray(x, dtype=np.float32)
    shared = {
        "w_qkv": np.ascontiguousarray(W_qkv, dtype=np.float32),
        "b_qkv": np.ascontiguousarray(b_qkv, dtype=np.float32),
        "w_out": np.ascontiguousarray(W_out, dtype=np.float32),
        "b_out": np.ascontiguousarray(b_out, dtype=np.float32),
    }
    in_maps = []
    for c in range(N_CORES):
        b, h = divmod(c, 2)
        xp = np.ascontiguousarray(x[b, h * SH : (h + 1) * SH])
        in_maps.append({"x": xp, **shared})

    import os
    import tempfile
    import time

    tmpdir = os.environ.get("ATTN_TRACE_DIR") or tempfile.mkdtemp(prefix="attn_trace_")
    res = None
    for attempt in range(3):
        try:
            res = run_bass_kernel_spmd(
                nc, in_maps, core_ids=list(range(N_CORES)), tmpdir=tmpdir
            )
            break
        except Exception:
            # transient NRT_EXEC_UNIT_UNRECOVERABLE has been observed on a
            # first attempt; a clean retry recovers
            if attempt == 2:
                raise
            time.sleep(2.0)
    global last_results, last_tmpdir
    last_results = res
    last_tmpdir = tmpdir

    out = np.empty((B, S, E), dtype=np.float32)
    for c in range(N_CORES):
        b, h = divmod(c, 2)
        out[b, h * SH : (h + 1) * SH] = res.results[c]["out"]
    return out


# revision 16
# speedup vs baseline: 1.0714x; 1.0244x over previous
"""Single-head attention (B=4, S=4096, E=1024, H=64) on 8 TRN2 NeuronCores.

Sharding: core c -> (batch b = c//2, sequence half h = c%2). Each core receives
only its own 2048-row x half, computes Q/K/V for it, and the core pair
(2b, 2b+1) exchanges K/V halves with a 2-rank AllGather (two chunked AGs,
overlapped with the projection and the first attention tiles). Every core then
holds K/V for the full 4096-row sequence in global order and computes
attention for its 2048 queries.

Matmuls run in bf16 (fp32 lowers to two LOW_HIGH PE passes on TRN2 — half
throughput); accumulation is fp32 in PSUM, the softmax denominator and the
normalization stay fp32. All matmuls are zero-padded to full 128x128
stationary tiles: masked sub-tile matmuls (K=64 / M=65) leave the PE
clock-gated at 1.2 GHz (HAM does not see them as activity), while full tiles
keep it at 2.4 GHz; the padding costs no extra stream cycles.

Output projection: W_out is padded with b_out as row 64 and the bf16 context
carries the softmax denominator in row 64, so (ctx_aug.T @ W_out_aug) *
recip(denom) applies scale and bias in one pass (denom * recip == 1).

Changes over the original two-phase version, from trace analysis:
- the phase-B constants (kt zero rows, v_aug ones, W_out staging) are emitted
  BEFORE the AllGather readouts on the gpsimd queue: they used to sit behind
  the readout that waits for the last AG (~80us), gating the first scores
  matmul at ~83us; now attention starts as soon as AG0's readout lands.
- the second query chunk runs attention+output-projection in two 512-wide
  subchunks: the exposed serial tail after the last context matmul (which ran
  at 1.2 GHz because the HAM clock gate re-throttles during the ~4us scalar
  chain) shrinks by half.
- the scalar (ACT) engine does only exp in the attention region; casts,
  PSUM copies, and output scaling run on DVE so exp is never queued behind
  them (exp is the attention-phase rate limiter at ~1.15us per [128,1024]).
"""

import sys

import numpy as np

for _p in ("/opt/trn_rl_repo",):
    if _p not in sys.path:
        sys.path.insert(0, _p)

from contextlib import ExitStack

import concourse.bass as bass  # noqa: F401  (import keeps bass registered)
import concourse.mybir as mybir
import concourse.tile as tile
from concourse import bacc, masks
from concourse.bass_utils import run_bass_kernel_spmd

F32 = mybir.dt.float32
BF16 = mybir.dt.bfloat16
AF = mybir.ActivationFunctionType
ALU = mybir.AluOpType

B, S, E, H = 4, 4096, 1024, 64
D3 = 3 * H            # 192
SH = S // 2           # queries per core
N_CORES = 8
QC = 1024             # first query chunk (PSUM-sized)
ST = S // 128         # 32 kj tiles over the full sequence
ETILES = E // 128     # 8 embedding tiles
WSTR = 256            # w_sb per-e-tile stride: [K|Q|V|0] columns
CW = 512              # phase-A chunk width (columns of the own half)
NCH = SH // CW        # 4 chunks, one AllGather each
SCALE = 0.125         # 1/sqrt(H)
WKV = 64 * CW * 2     # AG payload elems per chunk: kT[64,CW] + vT[64,CW]
REPLICA_GROUPS = [[0, 1], [2, 3], [4, 5], [6, 7]]


def _emit(nc, tc, x_ext, wq_ext, bq_ext, wo_ext, bo_ext, out_ext):
    with ExitStack() as top:
        const = top.enter_context(tc.tile_pool(name="const", bufs=1))

        # Critical path first: identity (needed by the first transposes) and
        # the QKV weight staging.
        ident = const.tile([128, 128], BF16)
        masks.make_identity(nc, ident[:])

        # Weights: DMA fp32 staging -> cast to bf16.
        wstage_ctx = ExitStack()
        wstage = wstage_ctx.enter_context(tc.tile_pool(name="wstage", bufs=1))
        w32 = wstage.tile([128, ETILES * D3], F32)
        nc.gpsimd.dma_start(
            w32[:].rearrange("p (e d) -> p e d", d=D3),
            wq_ext.rearrange("(e p) d -> p e d", p=128),
        )
        w_sb = const.tile([128, ETILES * WSTR], BF16)
        w_sb_v = w_sb[:].rearrange("p (e c) -> p e c", c=WSTR)
        w32_v = w32[:].rearrange("p (e c) -> p e c", c=D3)
        nc.vector.tensor_copy(w_sb_v[:, :, 0:64], w32_v[:, :, 64:128])     # K
        nc.vector.tensor_copy(w_sb_v[:, :, 64:128], w32_v[:, :, 0:64])     # Q
        nc.vector.tensor_copy(w_sb_v[:, :, 128:192], w32_v[:, :, 128:192])  # V

        # v_aug fully zeroed on DVE so ctx PSUM rows 65:128 are exact zeros
        # (enables a single full-width bf16 context copy in phase C)
        v_aug = const.tile([128, ST * 128], BF16)
        nc.vector.memset(v_aug[:], 0.0)
        nc.vector.memset(
            v_aug[:].rearrange("p (t c) -> p t c", c=128)[:, :, 64:65], 1.0
        )

        bkq = const.tile([128, 1], F32)  # [b_k ; b_q]
        nc.gpsimd.dma_start(bkq[0:64, :], bq_ext[64:128].unsqueeze(1))
        nc.gpsimd.dma_start(bkq[64:128, :], bq_ext[0:64].unsqueeze(1))
        bv = const.tile([64, 1], F32)
        nc.gpsimd.dma_start(bv[:], bq_ext[128:192].unsqueeze(1))

        # Persistent bf16 matmul operands (global kv order on the free axis)
        kt_sb = const.tile([128, S], BF16)   # kT on 0:64, zeros on 64:128
        vT_sb = const.tile([64, S], BF16)
        q2_sb = const.tile([128, SH], BF16)  # qT on 0:64, zeros on 64:128
        nc.gpsimd.memset(q2_sb[:], 0.0)
        ones11 = const.tile([1, 1], BF16)
        wo_sb = const.tile([128, E], BF16)

        def emit_consts():
            # phase-B constants on gpsimd: emitted between AG issues -- late
            # enough not to delay AG0's trigger, still ordered before the AG
            # readouts that gate the first scores matmul
            nc.gpsimd.memset(kt_sb[64:128, :], 0.0)
            nc.gpsimd.memset(ones11[:], 1.0)
            wo32 = wstage.tile([H, E], F32)
            nc.gpsimd.dma_start(wo32[:], wo_ext[:, :])
            bo32 = wstage.tile([1, E], F32)
            nc.gpsimd.dma_start(bo32[:], bo_ext.unsqueeze(0))
            bo16 = wstage.tile([1, E], BF16)
            nc.vector.tensor_copy(bo16[:], bo32[:])
            nc.gpsimd.memset(wo_sb[:], 0.0)
            nc.vector.tensor_copy(wo_sb[0:64, :], wo32[:])
            nc.gpsimd.dma_start(wo_sb[64:65, :], bo16[:])

        # Small PSUM pool shared by phase-A v-transposes and phase-C tiles
        mps = top.enter_context(tc.tile_pool(name="mps", bufs=2, space="PSUM"))

        # Collective bounce buffers (per AG chunk)
        dram = top.enter_context(tc.tile_pool(name="ccdram", bufs=1, space="DRAM"))
        cc_in = [dram.tile([1, WKV], BF16, name=f"cc_in{c}") for c in range(NCH)]
        cc_out = [dram.tile([2, WKV], BF16, name=f"cc_out{c}") for c in range(NCH)]

        # ---- Phase A: per own-half s-chunk: cast, PE-transpose, project,
        # stage K/V into the pair AllGather --------------------------------
        with ExitStack() as pa:
            xsb = pa.enter_context(tc.tile_pool(name="xsb", bufs=12))
            xbp = pa.enter_context(tc.tile_pool(name="xbp", bufs=8))
            xTp = pa.enter_context(tc.tile_pool(name="xTp", bufs=3))
            stg = pa.enter_context(tc.tile_pool(name="stg", bufs=5))
            kqs = []
            xtp = pa.enter_context(tc.tile_pool(name="xtp", bufs=2, space="PSUM"))
            m1p = pa.enter_context(tc.tile_pool(name="m1p", bufs=2, space="PSUM"))
            m2p = pa.enter_context(tc.tile_pool(name="m2p", bufs=2, space="PSUM"))

            for sc in range(NCH):              # own-half s chunks of CW cols
                xbs = []
                for si in range(CW // 128):
                    st = sc * (CW // 128) + si
                    t32 = xsb.tile([128, E], F32)
                    # split each tile across both HWDGE queues and cast each
                    # half as soon as it lands
                    nc.sync.dma_start(
                        t32[0:64, :], x_ext[st * 128 : st * 128 + 64, :]
                    )
                    nc.scalar.dma_start(
                        t32[64:128, :], x_ext[st * 128 + 64 : (st + 1) * 128, :]
                    )
                    tb = xbp.tile([128, E], BF16)
                    nc.vector.tensor_copy(tb[:], t32[:])
                    xbs.append(tb)
                xT_sc = xTp.tile([128, ETILES * CW], BF16)
                for e in range(ETILES):
                    p = xtp.tile([128, CW], F32)
                    for si in range(CW // 128):
                        nc.tensor.matmul(
                            p[:, si * 128 : (si + 1) * 128],
                            xbs[si][:, e * 128 : (e + 1) * 128],
                            ident[:],
                        )
                    nc.vector.tensor_copy(xT_sc[:, e * CW : (e + 1) * CW], p[:])

                m1 = m1p.tile([128, CW], F32)
                m2 = m2p.tile([128, CW], F32)
                for e in range(ETILES):
                    lhs1 = w_sb[:, e * WSTR : e * WSTR + 128]
                    lhs2 = w_sb[:, e * WSTR + 128 : e * WSTR + 256]
                    rhs = xT_sc[:, e * CW : (e + 1) * CW]
                    nc.tensor.matmul(
                        m1[:], lhs1, rhs,
                        start=(e == 0), stop=(e == ETILES - 1),
                    )
                    nc.tensor.matmul(
                        m2[:], lhs2, rhs,
                        start=(e == 0), stop=(e == ETILES - 1),
                    )
                kq = stg.tile([128, CW], BF16, tag="kq")
                kqs.append(kq)
                nc.vector.tensor_scalar_add(kq[:], m1[:], bkq[:])
                vst = stg.tile([64, CW], BF16, tag="vst")
                nc.vector.tensor_scalar_add(vst[:], m2[0:64, :], bv[:])

                # stage into the AG (gpsimd stream only)
                nc.gpsimd.dma_start(cc_in[sc][0, 0 : 64 * CW], kq[0:64, :])
                nc.gpsimd.dma_start(cc_in[sc][0, 64 * CW : WKV], vst[:])
                nc.gpsimd.collective_compute(
                    "AllGather",
                    ALU.bypass,
                    replica_groups=REPLICA_GROUPS,
                    ins=[cc_in[sc].opt()],
                    outs=[cc_out[sc].opt()],
                )
                if sc == 1:
                    emit_consts()

            # q2 moves and AG readouts AFTER every CC issue, all on the
            # gpsimd stream
            for sc in range(NCH):
                nc.gpsimd.dma_start(
                    q2_sb[0:64, sc * CW : (sc + 1) * CW], kqs[sc][64:128, :]
                )
            for sc in range(NCH):
                for r in range(2):
                    cols = slice(r * SH + sc * CW, r * SH + (sc + 1) * CW)
                    nc.gpsimd.dma_start(
                        kt_sb[0:64, cols],
                        cc_out[sc][r, 0 : 64 * CW].rearrange("(p f) -> p f", p=64),
                    )
                    nc.gpsimd.dma_start(
                        vT_sb[:, cols],
                        cc_out[sc][r, 64 * CW : WKV].rearrange("(p f) -> p f", p=64),
                    )
        wstage_ctx.close()

        # kj visit order: tiles in AG-chunk completion order
        kpc = CW // 128  # kj tiles per AG chunk per half
        kj_order = []
        for c in range(NCH):
            kj_order += list(range(c * kpc, (c + 1) * kpc))
            kj_order += list(range(16 + c * kpc, 16 + (c + 1) * kpc))

        # ---- Phase B/C: attention + output projection -------------------
        # first chunk at QC=1024 (best exp amortization), second chunk as
        # two 512-wide subchunks so the exposed end-of-kernel tail is short
        with ExitStack() as pb:
            sps = pb.enter_context(tc.tile_pool(name="sps", bufs=2, space="PSUM"))
            cps = pb.enter_context(tc.tile_pool(name="cps", bufs=1, space="PSUM"))
            expp = pb.enter_context(tc.tile_pool(name="expp", bufs=8))
            ctxp = pb.enter_context(tc.tile_pool(name="ctxp", bufs=2))
            rsp = pb.enter_context(tc.tile_pool(name="rsp", bufs=2))
            outp = pb.enter_context(tc.tile_pool(name="outp", bufs=4))

            first_visit = [True]

            def emit_attn(q0, w):
                """attention for queries [q0, q0+w); returns ctx PSUM tile"""
                ctx = cps.tile([128, QC], F32, name="ctx", tag="ctx")
                for i, kj in enumerate(kj_order):
                    sc_ps = sps.tile([128, QC], F32, name="sc_ps", tag="sc")
                    lhs_k = kt_sb[:, kj * 128 : (kj + 1) * 128]
                    for n in range(w // 512):
                        nc.tensor.matmul(
                            sc_ps[:, n * 512 : (n + 1) * 512],
                            lhs_k,
                            q2_sb[:, q0 + n * 512 : q0 + (n + 1) * 512],
                        )
                    ex = expp.tile([128, QC], BF16, name="ex", tag="ex")
                    nc.scalar.activation(
                        ex[:, 0:w], sc_ps[:, 0:w], AF.Exp, scale=SCALE
                    )
                    if first_visit[0]:  # v natural tile, first use
                        p = mps.tile([128, 64], F32, tag="mp", name="vtp")
                        nc.tensor.matmul(
                            p[:],
                            vT_sb[:, kj * 128 : (kj + 1) * 128],
                            ident[0:64, 0:64],
                        )
                        nc.vector.tensor_copy(
                            v_aug[:, kj * 128 : kj * 128 + 64], p[:]
                        )
                    lhs_v = v_aug[:, kj * 128 : (kj + 1) * 128]
                    for n in range(w // 512):
                        nc.tensor.matmul(
                            ctx[:, n * 512 : (n + 1) * 512],
                            lhs_v,
                            ex[:, n * 512 : (n + 1) * 512],
                            start=(i == 0), stop=(i == ST - 1),
                            skip_group_check=True,
                        )
                first_visit[0] = False
                return ctx

            def emit_phc(ctx, q0, w, last=False):
                """output projection for queries [q0, q0+w)"""
                # rows 65:128 of ctx are exact zeros (v_aug zero padding), so
                # one full-width bf16 copy suffices: the serial chain stays
                # under the ~3.4us HAM window and the out matmuls run warm
                ctx_b16 = ctxp.tile([128, QC], BF16, tag="ctx16", name="ctx_b16")
                nc.vector.tensor_copy(ctx_b16[:, 0:w], ctx[:, 0:w])
                rs_row = rsp.tile([1, QC], BF16, tag="rsrow", name="rs_row")
                nc.vector.tensor_copy(rs_row[:, 0:w], ctx_b16[64:65, 0:w])

                rs_ps = mps.tile([128, QC // 128], F32, tag="mp", name="rsps")
                for c in range(w // 128):
                    nc.tensor.matmul(
                        rs_ps[:, c : c + 1],
                        rs_row[0:1, c * 128 : (c + 1) * 128],
                        ones11[:],
                    )
                recip = rsp.tile([128, QC // 128], F32, tag="recip", name="recip")
                nc.vector.reciprocal(recip[:, 0 : w // 128], rs_ps[:, 0 : w // 128])

                for c in range(w // 128):
                    out_sb = outp.tile([128, E], F32, name="out_sb")
                    for n in range(2):
                        op = mps.tile([128, 512], F32, tag="mp", name="opps")
                        nc.tensor.matmul(
                            op[:],
                            ctx_b16[:, c * 128 : (c + 1) * 128],
                            wo_sb[:, n * 512 : (n + 1) * 512],
                        )
                        nc.vector.tensor_scalar_mul(
                            out_sb[:, n * 512 : (n + 1) * 512],
                            op[:],
                            recip[:, c : c + 1],
                        )
                    # mid-attention: sync only (a scalar-queue DMA issue
                    # costs ~0.6us of ACT time that would delay the exps);
                    # final chunk: both queues so the 4MB drain isn't serial
                    eng = nc.sync if (not last or c % 2 == 0) else nc.scalar
                    eng.dma_start(
                        out_ext[q0 + c * 128 : q0 + (c + 1) * 128, :], out_sb[:]
                    )

            ctx0 = emit_attn(0, 1024)
            emit_phc(ctx0, 0, 1024)
            ctx1 = emit_attn(1024, 1024)
            emit_phc(ctx1, 1024, 1024, last=True)


_NC = None


def _get_nc():
    global _NC
    if _NC is None:
        nc = bacc.Bacc("TRN2", target_bir_lowering=False, debug=False,
                       num_devices=N_CORES)
        x_ext = nc.dram_tensor("x", [SH, E], F32, kind="ExternalInput").ap()
        wq_ext = nc.dram_tensor("w_qkv", [E, D3], F32, kind="ExternalInput").ap()
        bq_ext = nc.dram_tensor("b_qkv", [D3], F32, kind="ExternalInput").ap()
        wo_ext = nc.dram_tensor("w_out", [H, E], F32, kind="ExternalInput").ap()
        bo_ext = nc.dram_tensor("b_out", [E], F32, kind="ExternalInput").ap()
        out_ext = nc.dram_tensor("out", [SH, E], F32, kind="ExternalOutput").ap()
        with tile.TileContext(nc) as tc:
            _emit(nc, tc, x_ext, wq_ext, bq_ext, wo_ext, bo_ext, out_ext)
        nc.compile()
        _NC = nc
    return _NC


last_results = None
last_tmpdir = None


def kernel(x, W_qkv, b_qkv, W_out, b_out):
    nc = _get_nc()
    x = np.ascontiguousarray(x, dtype=np.float32)
    shared = {
        "w_qkv": np.ascontiguousarray(W_qkv, dtype=np.float32),
        "b_qkv": np.ascontiguousarray(b_qkv, dtype=np.float32),
        "w_out": np.ascontiguousarray(W_out, dtype=np.float32),
        "b_out": np.ascontiguousarray(b_out, dtype=np.float32),
    }
    in_maps = []
    for c in range(N_CORES):
        b, h = divmod(c, 2)
        xp = np.ascontiguousarray(x[b, h * SH : (h + 1) * SH])
        in_maps.append({"x": xp, **shared})

    import os
    import tempfile
    import time

    tmpdir = os.environ.get("ATTN_TRACE_DIR") or tempfile.mkdtemp(prefix="attn_trace_")
    res = None
    for attempt in range(3):
        try:
            res = run_bass_kernel_spmd(
                nc, in_maps, core_ids=list(range(N_CORES)), tmpdir=tmpdir
            )
            break
        except Exception:
            # transient NRT_EXEC_UNIT_UNRECOVERABLE has been observed on a
            # first attempt; a clean retry recovers
            if attempt == 2:
                raise
            time.sleep(2.0)
    global last_results, last_tmpdir
    last_results = res
    last_tmpdir = tmpdir

    out = np.empty((B, S, E), dtype=np.float32)
    for c in range(N_CORES):
        b, h = divmod(c, 2)
        out[b, h * SH : (h + 1) * SH] = res.results[c]["out"]
    return out


# revision 17
# speedup vs baseline: 1.0836x; 1.0114x over previous
"""Single-head attention (B=4, S=4096, E=1024, H=64) on 8 TRN2 NeuronCores.

Sharding: core c -> (batch b = c//2, sequence half h = c%2). Each core receives
only its own 2048-row x half, computes Q/K/V for it, and the core pair
(2b, 2b+1) exchanges K/V halves with a 2-rank AllGather (two chunked AGs,
overlapped with the projection and the first attention tiles). Every core then
holds K/V for the full 4096-row sequence in global order and computes
attention for its 2048 queries.

Matmuls run in bf16 (fp32 lowers to two LOW_HIGH PE passes on TRN2 — half
throughput); accumulation is fp32 in PSUM, the softmax denominator and the
normalization stay fp32. All matmuls are zero-padded to full 128x128
stationary tiles: masked sub-tile matmuls (K=64 / M=65) leave the PE
clock-gated at 1.2 GHz (HAM does not see them as activity), while full tiles
keep it at 2.4 GHz; the padding costs no extra stream cycles.

Output projection: W_out is padded with b_out as row 64 and the bf16 context
carries the softmax denominator in row 64, so (ctx_aug.T @ W_out_aug) *
recip(denom) applies scale and bias in one pass (denom * recip == 1).

Changes over the original two-phase version, from trace analysis:
- the phase-B constants (kt zero rows, v_aug ones, W_out staging) are emitted
  BEFORE the AllGather readouts on the gpsimd queue: they used to sit behind
  the readout that waits for the last AG (~80us), gating the first scores
  matmul at ~83us; now attention starts as soon as AG0's readout lands.
- the second query chunk runs attention+output-projection in two 512-wide
  subchunks: the exposed serial tail after the last context matmul (which ran
  at 1.2 GHz because the HAM clock gate re-throttles during the ~4us scalar
  chain) shrinks by half.
- the scalar (ACT) engine does only exp in the attention region; casts,
  PSUM copies, and output scaling run on DVE so exp is never queued behind
  them (exp is the attention-phase rate limiter at ~1.15us per [128,1024]).
"""

import sys

import numpy as np

for _p in ("/opt/trn_rl_repo",):
    if _p not in sys.path:
        sys.path.insert(0, _p)

from contextlib import ExitStack

import concourse.bass as bass  # noqa: F401  (import keeps bass registered)
import concourse.mybir as mybir
import concourse.tile as tile
from concourse import bacc, masks
from concourse.bass_utils import run_bass_kernel_spmd

F32 = mybir.dt.float32
BF16 = mybir.dt.bfloat16
AF = mybir.ActivationFunctionType
ALU = mybir.AluOpType

B, S, E, H = 4, 4096, 1024, 64
D3 = 3 * H            # 192
SH = S // 2           # queries per core
N_CORES = 8
QC = 1024             # first query chunk (PSUM-sized)
ST = S // 128         # 32 kj tiles over the full sequence
ETILES = E // 128     # 8 embedding tiles
WSTR = 256            # w_sb per-e-tile stride: [K|Q|V|0] columns
CW = 512              # phase-A chunk width (columns of the own half)
NCH = SH // CW        # 4 chunks, one AllGather each
SCALE = 0.125         # 1/sqrt(H)
WKV = 64 * CW * 2     # AG payload elems per chunk: kT[64,CW] + vT[64,CW]
REPLICA_GROUPS = [[0, 1], [2, 3], [4, 5], [6, 7]]


def _emit(nc, tc, x_ext, wq_ext, bq_ext, wo_ext, bo_ext, out_ext):
    with ExitStack() as top:
        const = top.enter_context(tc.tile_pool(name="const", bufs=1))

        # Critical path first: identity (needed by the first transposes) and
        # the QKV weight staging.
        ident = const.tile([128, 128], BF16)
        masks.make_identity(nc, ident[:])

        # Weights: DMA fp32 staging -> cast to bf16.
        wstage_ctx = ExitStack()
        wstage = wstage_ctx.enter_context(tc.tile_pool(name="wstage", bufs=1))
        w32 = wstage.tile([128, ETILES * D3], F32)
        nc.gpsimd.dma_start(
            w32[:].rearrange("p (e d) -> p e d", d=D3),
            wq_ext.rearrange("(e p) d -> p e d", p=128),
        )
        w_sb = const.tile([128, ETILES * WSTR], BF16)
        w_sb_v = w_sb[:].rearrange("p (e c) -> p e c", c=WSTR)
        w32_v = w32[:].rearrange("p (e c) -> p e c", c=D3)
        nc.vector.tensor_copy(w_sb_v[:, :, 0:64], w32_v[:, :, 64:128])     # K
        nc.vector.tensor_copy(w_sb_v[:, :, 64:128], w32_v[:, :, 0:64])     # Q
        nc.vector.tensor_copy(w_sb_v[:, :, 128:192], w32_v[:, :, 128:192])  # V

        # v_aug fully zeroed on DVE so ctx PSUM rows 65:128 are exact zeros
        # (enables a single full-width bf16 context copy in phase C)
        v_aug = const.tile([128, ST * 128], BF16)
        nc.vector.memset(v_aug[:], 0.0)
        nc.vector.memset(
            v_aug[:].rearrange("p (t c) -> p t c", c=128)[:, :, 64:65], 1.0
        )

        bkq = const.tile([128, 1], F32)  # [b_k ; b_q]
        nc.gpsimd.dma_start(bkq[0:64, :], bq_ext[64:128].unsqueeze(1))
        nc.gpsimd.dma_start(bkq[64:128, :], bq_ext[0:64].unsqueeze(1))
        bv = const.tile([64, 1], F32)
        nc.gpsimd.dma_start(bv[:], bq_ext[128:192].unsqueeze(1))

        # Persistent bf16 matmul operands (global kv order on the free axis)
        kt_sb = const.tile([128, S], BF16)   # kT on 0:64, zeros on 64:128
        vT_sb = const.tile([64, S], BF16)
        q2_sb = const.tile([128, SH], BF16)  # qT on 0:64, zeros on 64:128
        nc.gpsimd.memset(q2_sb[:], 0.0)
        ones11 = const.tile([1, 1], BF16)
        wo_sb = const.tile([128, E], BF16)

        def emit_consts():
            # phase-B constants on gpsimd: emitted between AG issues -- late
            # enough not to delay AG0's trigger, still ordered before the AG
            # readouts that gate the first scores matmul
            nc.gpsimd.memset(kt_sb[64:128, :], 0.0)
            nc.gpsimd.memset(ones11[:], 1.0)
            wo32 = wstage.tile([H, E], F32)
            nc.gpsimd.dma_start(wo32[:], wo_ext[:, :])
            bo32 = wstage.tile([1, E], F32)
            nc.gpsimd.dma_start(bo32[:], bo_ext.unsqueeze(0))
            bo16 = wstage.tile([1, E], BF16)
            nc.vector.tensor_copy(bo16[:], bo32[:])
            nc.gpsimd.memset(wo_sb[:], 0.0)
            nc.vector.tensor_copy(wo_sb[0:64, :], wo32[:])
            nc.gpsimd.dma_start(wo_sb[64:65, :], bo16[:])

        # Small PSUM pool shared by phase-A v-transposes and phase-C tiles
        mps = top.enter_context(tc.tile_pool(name="mps", bufs=2, space="PSUM"))

        # Collective bounce buffers (per AG chunk)
        dram = top.enter_context(tc.tile_pool(name="ccdram", bufs=1, space="DRAM"))
        cc_in = [dram.tile([1, WKV], BF16, name=f"cc_in{c}") for c in range(NCH)]
        cc_out = [dram.tile([2, WKV], BF16, name=f"cc_out{c}") for c in range(NCH)]

        # ---- Phase A: per own-half s-chunk: cast, PE-transpose, project,
        # stage K/V into the pair AllGather --------------------------------
        with ExitStack() as pa:
            xsb = pa.enter_context(tc.tile_pool(name="xsb", bufs=12))
            xbp = pa.enter_context(tc.tile_pool(name="xbp", bufs=8))
            xTp = pa.enter_context(tc.tile_pool(name="xTp", bufs=3))
            stg = pa.enter_context(tc.tile_pool(name="stg", bufs=5))
            kqs = []
            xtp = pa.enter_context(tc.tile_pool(name="xtp", bufs=2, space="PSUM"))
            m1p = pa.enter_context(tc.tile_pool(name="m1p", bufs=2, space="PSUM"))
            m2p = pa.enter_context(tc.tile_pool(name="m2p", bufs=2, space="PSUM"))

            for sc in range(NCH):              # own-half s chunks of CW cols
                xbs = []
                for si in range(CW // 128):
                    st = sc * (CW // 128) + si
                    t32 = xsb.tile([128, E], F32)
                    # split each tile across both HWDGE queues and cast each
                    # half as soon as it lands
                    nc.sync.dma_start(
                        t32[0:64, :], x_ext[st * 128 : st * 128 + 64, :]
                    )
                    nc.scalar.dma_start(
                        t32[64:128, :], x_ext[st * 128 + 64 : (st + 1) * 128, :]
                    )
                    tb = xbp.tile([128, E], BF16)
                    nc.vector.tensor_copy(tb[:], t32[:])
                    xbs.append(tb)
                xT_sc = xTp.tile([128, ETILES * CW], BF16)
                for e in range(ETILES):
                    p = xtp.tile([128, CW], F32)
                    for si in range(CW // 128):
                        nc.tensor.matmul(
                            p[:, si * 128 : (si + 1) * 128],
                            xbs[si][:, e * 128 : (e + 1) * 128],
                            ident[:],
                        )
                    nc.vector.tensor_copy(xT_sc[:, e * CW : (e + 1) * CW], p[:])

                m1 = m1p.tile([128, CW], F32)
                m2 = m2p.tile([128, CW], F32)
                for e in range(ETILES):
                    lhs1 = w_sb[:, e * WSTR : e * WSTR + 128]
                    lhs2 = w_sb[:, e * WSTR + 128 : e * WSTR + 256]
                    rhs = xT_sc[:, e * CW : (e + 1) * CW]
                    nc.tensor.matmul(
                        m1[:], lhs1, rhs,
                        start=(e == 0), stop=(e == ETILES - 1),
                    )
                    nc.tensor.matmul(
                        m2[:], lhs2, rhs,
                        start=(e == 0), stop=(e == ETILES - 1),
                    )
                kq = stg.tile([128, CW], BF16, tag="kq")
                kqs.append(kq)
                nc.vector.tensor_scalar_add(kq[:], m1[:], bkq[:])
                vst = stg.tile([64, CW], BF16, tag="vst")
                nc.vector.tensor_scalar_add(vst[:], m2[0:64, :], bv[:])

                # stage into the AG (gpsimd stream only)
                nc.gpsimd.dma_start(cc_in[sc][0, 0 : 64 * CW], kq[0:64, :])
                nc.gpsimd.dma_start(cc_in[sc][0, 64 * CW : WKV], vst[:])
                nc.gpsimd.collective_compute(
                    "AllGather",
                    ALU.bypass,
                    replica_groups=REPLICA_GROUPS,
                    ins=[cc_in[sc].opt()],
                    outs=[cc_out[sc].opt()],
                )
                if sc == 1:
                    emit_consts()

            # q2 moves and AG readouts AFTER every CC issue, all on the
            # gpsimd stream
            for sc in range(NCH):
                nc.gpsimd.dma_start(
                    q2_sb[0:64, sc * CW : (sc + 1) * CW], kqs[sc][64:128, :]
                )
            for sc in range(NCH):
                for r in range(2):
                    cols = slice(r * SH + sc * CW, r * SH + (sc + 1) * CW)
                    nc.gpsimd.dma_start(
                        kt_sb[0:64, cols],
                        cc_out[sc][r, 0 : 64 * CW].rearrange("(p f) -> p f", p=64),
                    )
                    nc.gpsimd.dma_start(
                        vT_sb[:, cols],
                        cc_out[sc][r, 64 * CW : WKV].rearrange("(p f) -> p f", p=64),
                    )
        wstage_ctx.close()

        # kj visit order: tiles in AG-chunk completion order
        kpc = CW // 128  # kj tiles per AG chunk per half
        kj_order = []
        for c in range(NCH):
            kj_order += list(range(c * kpc, (c + 1) * kpc))
            kj_order += list(range(16 + c * kpc, 16 + (c + 1) * kpc))

        # ---- Phase B/C: attention + output projection -------------------
        # first chunk at QC=1024 (best exp amortization), second chunk as
        # two 512-wide subchunks so the exposed end-of-kernel tail is short
        with ExitStack() as pb:
            sps = pb.enter_context(tc.tile_pool(name="sps", bufs=2, space="PSUM"))
            cps = pb.enter_context(tc.tile_pool(name="cps", bufs=1, space="PSUM"))
            expp = pb.enter_context(tc.tile_pool(name="expp", bufs=8))
            ctxp = pb.enter_context(tc.tile_pool(name="ctxp", bufs=2))
            rsp = pb.enter_context(tc.tile_pool(name="rsp", bufs=2))
            outp = pb.enter_context(tc.tile_pool(name="outp", bufs=4))

            first_visit = [True]

            def emit_attn(ctx, q0, w, lo, hi):
                """attention for queries [q0, q0+w), kj_order[lo:hi]"""
                for i in range(lo, hi):
                    kj = kj_order[i]
                    sc_ps = sps.tile([128, QC], F32, name="sc_ps", tag="sc")
                    lhs_k = kt_sb[:, kj * 128 : (kj + 1) * 128]
                    for n in range(w // 512):
                        nc.tensor.matmul(
                            sc_ps[:, n * 512 : (n + 1) * 512],
                            lhs_k,
                            q2_sb[:, q0 + n * 512 : q0 + (n + 1) * 512],
                        )
                    ex = expp.tile([128, QC], BF16, name="ex", tag="ex")
                    nc.scalar.activation(
                        ex[:, 0:w], sc_ps[:, 0:w], AF.Exp, scale=SCALE
                    )
                    if first_visit[0]:  # v natural tile, first use
                        p = mps.tile([128, 64], F32, tag="mp", name="vtp")
                        nc.tensor.matmul(
                            p[:],
                            vT_sb[:, kj * 128 : (kj + 1) * 128],
                            ident[0:64, 0:64],
                        )
                        nc.vector.tensor_copy(
                            v_aug[:, kj * 128 : kj * 128 + 64], p[:]
                        )
                    lhs_v = v_aug[:, kj * 128 : (kj + 1) * 128]
                    for n in range(w // 512):
                        nc.tensor.matmul(
                            ctx[:, n * 512 : (n + 1) * 512],
                            lhs_v,
                            ex[:, n * 512 : (n + 1) * 512],
                            start=(i == 0), stop=(i == ST - 1),
                            skip_group_check=True,
                        )
                if hi == ST:
                    first_visit[0] = False

            def emit_phc_pre(ctx, q0, w):
                """vector-only context handoff: frees the ctx PSUM bank"""
                # rows 65:128 of ctx are exact zeros (v_aug zero padding), so
                # one full-width bf16 copy suffices
                ctx_b16 = ctxp.tile([128, QC], BF16, tag="ctx16", name="ctx_b16")
                nc.vector.tensor_copy(ctx_b16[:, 0:w], ctx[:, 0:w])
                rs_row = rsp.tile([1, QC], BF16, tag="rsrow", name="rs_row")
                nc.vector.tensor_copy(rs_row[:, 0:w], ctx_b16[64:65, 0:w])
                return ctx_b16, rs_row

            def emit_phc_out(ctx_b16, rs_row, q0, w, last=False):
                """rs transpose + recip + output matmuls; deferred into the
                next chunk's attention stream so the PE's per-kj slack absorbs
                it and the ACT exp stream never idles"""

                rs_ps = mps.tile([128, QC // 128], F32, tag="mp", name="rsps")
                for c in range(w // 128):
                    nc.tensor.matmul(
                        rs_ps[:, c : c + 1],
                        rs_row[0:1, c * 128 : (c + 1) * 128],
                        ones11[:],
                    )
                recip = rsp.tile([128, QC // 128], F32, tag="recip", name="recip")
                nc.vector.reciprocal(recip[:, 0 : w // 128], rs_ps[:, 0 : w // 128])

                for c in range(w // 128):
                    out_sb = outp.tile([128, E], F32, name="out_sb")
                    for n in range(2):
                        op = mps.tile([128, 512], F32, tag="mp", name="opps")
                        nc.tensor.matmul(
                            op[:],
                            ctx_b16[:, c * 128 : (c + 1) * 128],
                            wo_sb[:, n * 512 : (n + 1) * 512],
                        )
                        nc.vector.tensor_scalar_mul(
                            out_sb[:, n * 512 : (n + 1) * 512],
                            op[:],
                            recip[:, c : c + 1],
                        )
                    # mid-attention: sync only (a scalar-queue DMA issue
                    # costs ~0.6us of ACT time that would delay the exps);
                    # final chunk: both queues so the 4MB drain isn't serial
                    eng = nc.sync if (not last or c % 2 == 0) else nc.scalar
                    eng.dma_start(
                        out_ext[q0 + c * 128 : q0 + (c + 1) * 128, :], out_sb[:]
                    )

            ctx0 = cps.tile([128, QC], F32, name="ctx", tag="ctx")
            emit_attn(ctx0, 0, 1024, 0, ST)
            h0 = emit_phc_pre(ctx0, 0, 1024)
            ctx1 = cps.tile([128, QC], F32, name="ctx", tag="ctx")
            emit_attn(ctx1, 1024, 1024, 0, 8)
            emit_phc_out(*h0, 0, 1024)
            emit_attn(ctx1, 1024, 1024, 8, ST)
            h1 = emit_phc_pre(ctx1, 1024, 1024)
            emit_phc_out(*h1, 1024, 1024, last=True)


_NC = None


def _get_nc():
    global _NC
    if _NC is None:
        nc = bacc.Bacc("TRN2", target_bir_lowering=False, debug=False,
                       num_devices=N_CORES)
        x_ext = nc.dram_tensor("x", [SH, E], F32, kind="ExternalInput").ap()
        wq_ext = nc.dram_tensor("w_qkv", [E, D3], F32, kind="ExternalInput").ap()
        bq_ext = nc.dram_tensor("b_qkv", [D3], F32, kind="ExternalInput").ap()
        wo_ext = nc.dram_tensor("w_out", [H, E], F32, kind="ExternalInput").ap()
        bo_ext = nc.dram_tensor("b_out", [E], F32, kind="ExternalInput").ap()
        out_ext = nc.dram_tensor("out", [SH, E], F32, kind="ExternalOutput").ap()
        with tile.TileContext(nc) as tc:
            _emit(nc, tc, x_ext, wq_ext, bq_ext, wo_ext, bo_ext, out_ext)
        nc.compile()
        _NC = nc
    return _NC


last_results = None
last_tmpdir = None


def kernel(x, W_qkv, b_qkv, W_out, b_out):
    nc = _get_nc()
    x = np.ascontiguousarray(x, dtype=np.float32)
    shared = {
        "w_qkv": np.ascontiguousarray(W_qkv, dtype=np.float32),
        "b_qkv": np.ascontiguousarray(b_qkv, dtype=np.float32),
        "w_out": np.ascontiguousarray(W_out, dtype=np.float32),
        "b_out": np.ascontiguousarray(b_out, dtype=np.float32),
    }
    in_maps = []
    for c in range(N_CORES):
        b, h = divmod(c, 2)
        xp = np.ascontiguousarray(x[b, h * SH : (h + 1) * SH])
        in_maps.append({"x": xp, **shared})

    import os
    import tempfile
    import time

    tmpdir = os.environ.get("ATTN_TRACE_DIR") or tempfile.mkdtemp(prefix="attn_trace_")
    res = None
    for attempt in range(3):
        try:
            res = run_bass_kernel_spmd(
                nc, in_maps, core_ids=list(range(N_CORES)), tmpdir=tmpdir
            )
            break
        except Exception:
            # transient NRT_EXEC_UNIT_UNRECOVERABLE has been observed on a
            # first attempt; a clean retry recovers
            if attempt == 2:
                raise
            time.sleep(2.0)
    global last_results, last_tmpdir
    last_results = res
    last_tmpdir = tmpdir

    out = np.empty((B, S, E), dtype=np.float32)
    for c in range(N_CORES):
        b, h = divmod(c, 2)
        out[b, h * SH : (h + 1) * SH] = res.results[c]["out"]
    return out


# revision 18
# speedup vs baseline: 1.0844x; 1.0007x over previous
"""Single-head attention (B=4, S=4096, E=1024, H=64) on 8 TRN2 NeuronCores.

Sharding: core c -> (batch b = c//2, sequence half h = c%2). Each core receives
only its own 2048-row x half, computes Q/K/V for it, and the core pair
(2b, 2b+1) exchanges K/V halves with a 2-rank AllGather (two chunked AGs,
overlapped with the projection and the first attention tiles). Every core then
holds K/V for the full 4096-row sequence in global order and computes
attention for its 2048 queries.

Matmuls run in bf16 (fp32 lowers to two LOW_HIGH PE passes on TRN2 — half
throughput); accumulation is fp32 in PSUM, the softmax denominator and the
normalization stay fp32. All matmuls are zero-padded to full 128x128
stationary tiles: masked sub-tile matmuls (K=64 / M=65) leave the PE
clock-gated at 1.2 GHz (HAM does not see them as activity), while full tiles
keep it at 2.4 GHz; the padding costs no extra stream cycles.

Output projection: W_out is padded with b_out as row 64 and the bf16 context
carries the softmax denominator in row 64, so (ctx_aug.T @ W_out_aug) *
recip(denom) applies scale and bias in one pass (denom * recip == 1).

Changes over the original two-phase version, from trace analysis:
- the phase-B constants (kt zero rows, v_aug ones, W_out staging) are emitted
  BEFORE the AllGather readouts on the gpsimd queue: they used to sit behind
  the readout that waits for the last AG (~80us), gating the first scores
  matmul at ~83us; now attention starts as soon as AG0's readout lands.
- the second query chunk runs attention+output-projection in two 512-wide
  subchunks: the exposed serial tail after the last context matmul (which ran
  at 1.2 GHz because the HAM clock gate re-throttles during the ~4us scalar
  chain) shrinks by half.
- the scalar (ACT) engine does only exp in the attention region; casts,
  PSUM copies, and output scaling run on DVE so exp is never queued behind
  them (exp is the attention-phase rate limiter at ~1.15us per [128,1024]).
"""

import sys

import numpy as np

for _p in ("/opt/trn_rl_repo",):
    if _p not in sys.path:
        sys.path.insert(0, _p)

from contextlib import ExitStack

import concourse.bass as bass  # noqa: F401  (import keeps bass registered)
import concourse.mybir as mybir
import concourse.tile as tile
from concourse import bacc, masks
from concourse.bass_utils import run_bass_kernel_spmd

F32 = mybir.dt.float32
BF16 = mybir.dt.bfloat16
AF = mybir.ActivationFunctionType
ALU = mybir.AluOpType

B, S, E, H = 4, 4096, 1024, 64
D3 = 3 * H            # 192
SH = S // 2           # queries per core
N_CORES = 8
QC = 1024             # first query chunk (PSUM-sized)
ST = S // 128         # 32 kj tiles over the full sequence
ETILES = E // 128     # 8 embedding tiles
WSTR = 256            # w_sb per-e-tile stride: [K|Q|V|0] columns
CW = 512              # phase-A chunk width (columns of the own half)
NCH = SH // CW        # 4 chunks, one AllGather each
SCALE = 0.125         # 1/sqrt(H)
WKV = 64 * CW * 2     # AG payload elems per chunk: kT[64,CW] + vT[64,CW]
REPLICA_GROUPS = [[0, 1], [2, 3], [4, 5], [6, 7]]


def _emit(nc, tc, x_ext, wq_ext, bq_ext, wo_ext, bo_ext, out_ext):
    with ExitStack() as top:
        const = top.enter_context(tc.tile_pool(name="const", bufs=1))

        # Critical path first: identity (needed by the first transposes) and
        # the QKV weight staging.
        ident = const.tile([128, 128], BF16)
        masks.make_identity(nc, ident[:])

        # Weights: DMA fp32 staging -> cast to bf16.
        wstage_ctx = ExitStack()
        wstage = wstage_ctx.enter_context(tc.tile_pool(name="wstage", bufs=1))
        w32 = wstage.tile([128, ETILES * D3], F32)
        nc.gpsimd.dma_start(
            w32[:].rearrange("p (e d) -> p e d", d=D3),
            wq_ext.rearrange("(e p) d -> p e d", p=128),
        )
        w_sb = const.tile([128, ETILES * WSTR], BF16)
        w_sb_v = w_sb[:].rearrange("p (e c) -> p e c", c=WSTR)
        w32_v = w32[:].rearrange("p (e c) -> p e c", c=D3)
        nc.vector.tensor_copy(w_sb_v[:, :, 0:64], w32_v[:, :, 64:128])     # K
        nc.vector.tensor_copy(w_sb_v[:, :, 64:128], w32_v[:, :, 0:64])     # Q
        nc.vector.tensor_copy(w_sb_v[:, :, 128:192], w32_v[:, :, 128:192])  # V

        # v_aug fully zeroed on DVE so ctx PSUM rows 65:128 are exact zeros
        # (enables a single full-width bf16 context copy in phase C)
        v_aug = const.tile([128, ST * 128], BF16)
        nc.vector.memset(v_aug[:], 0.0)
        nc.vector.memset(
            v_aug[:].rearrange("p (t c) -> p t c", c=128)[:, :, 64:65], 1.0
        )

        bkq = const.tile([128, 1], F32)  # [b_k ; b_q]
        nc.gpsimd.dma_start(bkq[0:64, :], bq_ext[64:128].unsqueeze(1))
        nc.gpsimd.dma_start(bkq[64:128, :], bq_ext[0:64].unsqueeze(1))
        bv = const.tile([64, 1], F32)
        nc.gpsimd.dma_start(bv[:], bq_ext[128:192].unsqueeze(1))

        # Persistent bf16 matmul operands (global kv order on the free axis)
        kt_sb = const.tile([128, S], BF16)   # kT on 0:64, zeros on 64:128
        vT_sb = const.tile([64, S], BF16)
        q2_sb = const.tile([128, SH], BF16)  # qT on 0:64, zeros on 64:128
        nc.gpsimd.memset(q2_sb[:], 0.0)
        ones11 = const.tile([1, 1], BF16)
        wo_sb = const.tile([128, E], BF16)

        def emit_consts():
            # phase-B constants on gpsimd: emitted between AG issues -- late
            # enough not to delay AG0's trigger, still ordered before the AG
            # readouts that gate the first scores matmul
            nc.gpsimd.memset(kt_sb[64:128, :], 0.0)
            nc.gpsimd.memset(ones11[:], 1.0)
            wo32 = wstage.tile([H, E], F32)
            nc.gpsimd.dma_start(wo32[:], wo_ext[:, :])
            bo32 = wstage.tile([1, E], F32)
            nc.gpsimd.dma_start(bo32[:], bo_ext.unsqueeze(0))
            bo16 = wstage.tile([1, E], BF16)
            nc.vector.tensor_copy(bo16[:], bo32[:])
            nc.gpsimd.memset(wo_sb[:], 0.0)
            nc.vector.tensor_copy(wo_sb[0:64, :], wo32[:])
            nc.gpsimd.dma_start(wo_sb[64:65, :], bo16[:])

        # Small PSUM pool shared by phase-A v-transposes and phase-C tiles
        mps = top.enter_context(tc.tile_pool(name="mps", bufs=2, space="PSUM"))

        # Collective bounce buffers (per AG chunk)
        dram = top.enter_context(tc.tile_pool(name="ccdram", bufs=1, space="DRAM"))
        cc_in = [dram.tile([1, WKV], BF16, name=f"cc_in{c}") for c in range(NCH)]
        cc_out = [dram.tile([2, WKV], BF16, name=f"cc_out{c}") for c in range(NCH)]

        # ---- Phase A: per own-half s-chunk: cast, PE-transpose, project,
        # stage K/V into the pair AllGather --------------------------------
        with ExitStack() as pa:
            xsb = pa.enter_context(tc.tile_pool(name="xsb", bufs=12))
            xbp = pa.enter_context(tc.tile_pool(name="xbp", bufs=8))
            xTp = pa.enter_context(tc.tile_pool(name="xTp", bufs=3))
            stg = pa.enter_context(tc.tile_pool(name="stg", bufs=5))
            kqs = []
            xtp = pa.enter_context(tc.tile_pool(name="xtp", bufs=2, space="PSUM"))
            m1p = pa.enter_context(tc.tile_pool(name="m1p", bufs=2, space="PSUM"))
            m2p = pa.enter_context(tc.tile_pool(name="m2p", bufs=2, space="PSUM"))

            for sc in range(NCH):              # own-half s chunks of CW cols
                xbs = []
                for si in range(CW // 128):
                    st = sc * (CW // 128) + si
                    t32 = xsb.tile([128, E], F32)
                    # split each tile across both HWDGE queues and cast each
                    # half as soon as it lands
                    nc.sync.dma_start(
                        t32[0:64, :], x_ext[st * 128 : st * 128 + 64, :]
                    )
                    nc.scalar.dma_start(
                        t32[64:128, :], x_ext[st * 128 + 64 : (st + 1) * 128, :]
                    )
                    tb = xbp.tile([128, E], BF16)
                    nc.vector.tensor_copy(tb[:], t32[:])
                    xbs.append(tb)
                xT_sc = xTp.tile([128, ETILES * CW], BF16)
                for e in range(ETILES):
                    p = xtp.tile([128, CW], F32)
                    for si in range(CW // 128):
                        nc.tensor.matmul(
                            p[:, si * 128 : (si + 1) * 128],
                            xbs[si][:, e * 128 : (e + 1) * 128],
                            ident[:],
                        )
                    nc.vector.tensor_copy(xT_sc[:, e * CW : (e + 1) * CW], p[:])

                m1 = m1p.tile([128, CW], F32)
                m2 = m2p.tile([128, CW], F32)
                for e in range(ETILES):
                    lhs1 = w_sb[:, e * WSTR : e * WSTR + 128]
                    lhs2 = w_sb[:, e * WSTR + 128 : e * WSTR + 256]
                    rhs = xT_sc[:, e * CW : (e + 1) * CW]
                    nc.tensor.matmul(
                        m1[:], lhs1, rhs,
                        start=(e == 0), stop=(e == ETILES - 1),
                    )
                    nc.tensor.matmul(
                        m2[:], lhs2, rhs,
                        start=(e == 0), stop=(e == ETILES - 1),
                    )
                kq = stg.tile([128, CW], BF16, tag="kq")
                kqs.append(kq)
                nc.vector.tensor_scalar_add(kq[:], m1[:], bkq[:])
                vst = stg.tile([64, CW], BF16, tag="vst")
                nc.vector.tensor_scalar_add(vst[:], m2[0:64, :], bv[:])

                # stage into the AG (gpsimd stream only)
                nc.gpsimd.dma_start(cc_in[sc][0, 0 : 64 * CW], kq[0:64, :])
                nc.gpsimd.dma_start(cc_in[sc][0, 64 * CW : WKV], vst[:])
                nc.gpsimd.collective_compute(
                    "AllGather",
                    ALU.bypass,
                    replica_groups=REPLICA_GROUPS,
                    ins=[cc_in[sc].opt()],
                    outs=[cc_out[sc].opt()],
                )
                if sc == 1:
                    emit_consts()

            # q2 moves and AG readouts AFTER every CC issue, all on the
            # gpsimd stream
            for sc in range(NCH):
                nc.gpsimd.dma_start(
                    q2_sb[0:64, sc * CW : (sc + 1) * CW], kqs[sc][64:128, :]
                )
            for sc in range(NCH):
                for r in range(2):
                    cols = slice(r * SH + sc * CW, r * SH + (sc + 1) * CW)
                    nc.gpsimd.dma_start(
                        kt_sb[0:64, cols],
                        cc_out[sc][r, 0 : 64 * CW].rearrange("(p f) -> p f", p=64),
                    )
                    nc.gpsimd.dma_start(
                        vT_sb[:, cols],
                        cc_out[sc][r, 64 * CW : WKV].rearrange("(p f) -> p f", p=64),
                    )
        wstage_ctx.close()

        # kj visit order: tiles in AG-chunk completion order
        kpc = CW // 128  # kj tiles per AG chunk per half
        kj_order = []
        for c in range(NCH):
            kj_order += list(range(c * kpc, (c + 1) * kpc))
            kj_order += list(range(16 + c * kpc, 16 + (c + 1) * kpc))

        # ---- Phase B/C: attention + output projection -------------------
        # first chunk at QC=1024 (best exp amortization), second chunk as
        # two 512-wide subchunks so the exposed end-of-kernel tail is short
        with ExitStack() as pb:
            sps = pb.enter_context(tc.tile_pool(name="sps", bufs=2, space="PSUM"))
            cps = pb.enter_context(tc.tile_pool(name="cps", bufs=1, space="PSUM"))
            expp = pb.enter_context(tc.tile_pool(name="expp", bufs=8))
            ctxp = pb.enter_context(tc.tile_pool(name="ctxp", bufs=2))
            rsp = pb.enter_context(tc.tile_pool(name="rsp", bufs=2))
            outp = pb.enter_context(tc.tile_pool(name="outp", bufs=4))

            first_visit = [True]

            def emit_attn(ctx, q0, w, lo, hi):
                """attention for queries [q0, q0+w), kj_order[lo:hi]"""
                for i in range(lo, hi):
                    kj = kj_order[i]
                    sc_ps = sps.tile([128, QC], F32, name="sc_ps", tag="sc")
                    lhs_k = kt_sb[:, kj * 128 : (kj + 1) * 128]
                    for n in range(w // 512):
                        nc.tensor.matmul(
                            sc_ps[:, n * 512 : (n + 1) * 512],
                            lhs_k,
                            q2_sb[:, q0 + n * 512 : q0 + (n + 1) * 512],
                        )
                    ex = expp.tile([128, QC], BF16, name="ex", tag="ex")
                    nc.scalar.activation(
                        ex[:, 0:w], sc_ps[:, 0:w], AF.Exp, scale=SCALE
                    )
                    if first_visit[0]:  # v natural tile, first use
                        p = mps.tile([128, 64], F32, tag="mp", name="vtp")
                        nc.tensor.matmul(
                            p[:],
                            vT_sb[:, kj * 128 : (kj + 1) * 128],
                            ident[0:64, 0:64],
                        )
                        nc.vector.tensor_copy(
                            v_aug[:, kj * 128 : kj * 128 + 64], p[:]
                        )
                    lhs_v = v_aug[:, kj * 128 : (kj + 1) * 128]
                    for n in range(w // 512):
                        nc.tensor.matmul(
                            ctx[:, n * 512 : (n + 1) * 512],
                            lhs_v,
                            ex[:, n * 512 : (n + 1) * 512],
                            start=(i == 0), stop=(i == ST - 1),
                            skip_group_check=True,
                        )
                if hi == ST:
                    first_visit[0] = False

            def emit_phc_pre(ctx, q0, w):
                """vector-only context handoff: frees the ctx PSUM bank"""
                # rows 65:128 of ctx are exact zeros (v_aug zero padding), so
                # one full-width bf16 copy suffices
                ctx_b16 = ctxp.tile([128, QC], BF16, tag="ctx16", name="ctx_b16")
                nc.vector.tensor_copy(ctx_b16[:, 0:w], ctx[:, 0:w])
                rs_row = rsp.tile([1, QC], BF16, tag="rsrow", name="rs_row")
                nc.vector.tensor_copy(rs_row[:, 0:w], ctx_b16[64:65, 0:w])
                return ctx_b16, rs_row

            def emit_phc_out(ctx_b16, rs_row, q0, w, last=False):
                """rs transpose + recip + output matmuls; deferred into the
                next chunk's attention stream so the PE's per-kj slack absorbs
                it and the ACT exp stream never idles"""
                if last:
                    # keep-warm: the rs matmuls are K=1 masked ops that the
                    # HAM activity monitor cannot see, so the serial chain
                    # here reads as PE-idle and the clock gate drops to
                    # 1.2 GHz for the final output matmuls. A few full-tile
                    # matmuls into a scratch bank hold it at 2.4 GHz.
                    for _ in range(4):
                        warm = mps.tile([128, 512], F32, tag="mp", name="warm")
                        nc.tensor.matmul(warm[:], ident[:], q2_sb[:, 0:512])
                rs_ps = mps.tile([128, QC // 128], F32, tag="mp", name="rsps")
                for c in range(w // 128):
                    nc.tensor.matmul(
                        rs_ps[:, c : c + 1],
                        rs_row[0:1, c * 128 : (c + 1) * 128],
                        ones11[:],
                    )
                recip = rsp.tile([128, QC // 128], F32, tag="recip", name="recip")
                nc.vector.reciprocal(recip[:, 0 : w // 128], rs_ps[:, 0 : w // 128])

                for c in range(w // 128):
                    out_sb = outp.tile([128, E], F32, name="out_sb")
                    for n in range(2):
                        op = mps.tile([128, 512], F32, tag="mp", name="opps")
                        nc.tensor.matmul(
                            op[:],
                            ctx_b16[:, c * 128 : (c + 1) * 128],
                            wo_sb[:, n * 512 : (n + 1) * 512],
                        )
                        nc.vector.tensor_scalar_mul(
                            out_sb[:, n * 512 : (n + 1) * 512],
                            op[:],
                            recip[:, c : c + 1],
                        )
                    # mid-attention: sync only (a scalar-queue DMA issue
                    # costs ~0.6us of ACT time that would delay the exps);
                    # final chunk: both queues so the 4MB drain isn't serial
                    eng = nc.sync if (not last or c % 2 == 0) else nc.scalar
                    eng.dma_start(
                        out_ext[q0 + c * 128 : q0 + (c + 1) * 128, :], out_sb[:]
                    )

            ctx0 = cps.tile([128, QC], F32, name="ctx", tag="ctx")
            emit_attn(ctx0, 0, 1024, 0, ST)
            h0 = emit_phc_pre(ctx0, 0, 1024)
            ctx1 = cps.tile([128, QC], F32, name="ctx", tag="ctx")
            emit_attn(ctx1, 1024, 1024, 0, 8)
            emit_phc_out(*h0, 0, 1024)
            emit_attn(ctx1, 1024, 1024, 8, ST)
            h1 = emit_phc_pre(ctx1, 1024, 1024)
            emit_phc_out(*h1, 1024, 1024, last=True)


_NC = None


def _get_nc():
    global _NC
    if _NC is None:
        nc = bacc.Bacc("TRN2", target_bir_lowering=False, debug=False,
                       num_devices=N_CORES)
        x_ext = nc.dram_tensor("x", [SH, E], F32, kind="ExternalInput").ap()
        wq_ext = nc.dram_tensor("w_qkv", [E, D3], F32, kind="ExternalInput").ap()
        bq_ext = nc.dram_tensor("b_qkv", [D3], F32, kind="ExternalInput").ap()
        wo_ext = nc.dram_tensor("w_out", [H, E], F32, kind="ExternalInput").ap()
        bo_ext = nc.dram_tensor("b_out", [E], F32, kind="ExternalInput").ap()
        out_ext = nc.dram_tensor("out", [SH, E], F32, kind="ExternalOutput").ap()
        with tile.TileContext(nc) as tc:
            _emit(nc, tc, x_ext, wq_ext, bq_ext, wo_ext, bo_ext, out_ext)
        nc.compile()
        _NC = nc
    return _NC


last_results = None
last_tmpdir = None


def kernel(x, W_qkv, b_qkv, W_out, b_out):
    nc = _get_nc()
    x = np.ascontiguousarray(x, dtype=np.float32)
    shared = {
        "w_qkv": np.ascontiguousarray(W_qkv, dtype=np.float32),
        "b_qkv": np.ascontiguousarray(b_qkv, dtype=np.float32),
        "w_out": np.ascontiguousarray(W_out, dtype=np.float32),
        "b_out": np.ascontiguousarray(b_out, dtype=np.float32),
    }
    in_maps = []
    for c in range(N_CORES):
        b, h = divmod(c, 2)
        xp = np.ascontiguousarray(x[b, h * SH : (h + 1) * SH])
        in_maps.append({"x": xp, **shared})

    import os
    import tempfile
    import time

    tmpdir = os.environ.get("ATTN_TRACE_DIR") or tempfile.mkdtemp(prefix="attn_trace_")
    res = None
    for attempt in range(3):
        try:
            res = run_bass_kernel_spmd(
                nc, in_maps, core_ids=list(range(N_CORES)), tmpdir=tmpdir
            )
            break
        except Exception:
            # transient NRT_EXEC_UNIT_UNRECOVERABLE has been observed on a
            # first attempt; a clean retry recovers
            if attempt == 2:
                raise
            time.sleep(2.0)
    global last_results, last_tmpdir
    last_results = res
    last_tmpdir = tmpdir

    out = np.empty((B, S, E), dtype=np.float32)
    for c in range(N_CORES):
        b, h = divmod(c, 2)
        out[b, h * SH : (h + 1) * SH] = res.results[c]["out"]
    return out
